# revision 9
# baseline (speedup 1.0000x reference)
"""Trainium2 Bass kernel: data-dependent radix-2 FFT butterfly network.

out = FFT-like transform of x (4096x4096 f32 -> complex64); stage twiddles
are exp(-2j*pi*k/N * weights[k, :]) (learned, per-feature), N = 4096,
12 radix-2 stages, initial row permutation j ^ N/2.

Sharding: feature dim split across 8 NeuronCores (512 each) - the whole
network is elementwise along features, so no cross-core communication.

Per-core: features on partitions (4 groups of 128), FFT rows along the
free dim, x stored as packed (re, im) fp16 pairs. Each generic stage is
3 Vector-engine ops: a packed-complex-multiply custom DVE op (one
complex/cycle in 2X_1PORT mode) plus packed fp16 add/subs in 2x mode;
part of the butterfly add/sub work is offloaded to the otherwise-idle
GpSimd engine each stage. Stages 1-2 (real inputs, trivial twiddles) are
four fused quad ops that write the packed complex layout directly.
Twiddles are generated on-device by the Scalar engine's Sin LUT from
host-range-reduced phases. I/O is fp16 end-to-end: the host pre-permutes
(j ^ N/2), transposes and converts x to fp16, and the packed fp16 output
is converted to complex64 on the host. All tile pools are double-
buffered so consecutive 128-feature groups overlap.
"""

import math
import sys

import numpy as np

if "/opt/trn_rl_repo" not in sys.path:
    sys.path.insert(0, "/opt/trn_rl_repo")

import concourse.bacc as bacc
import concourse.bass as bass
import concourse.mybir as mybir
from concourse.bass_utils import run_bass_kernel_spmd
from concourse.tile import TileContext

F32 = mybir.dt.float32
F16 = mybir.dt.float16
AF = mybir.ActivationFunctionType
ALU = mybir.AluOpType

N = 4096
LOGN = 12
NCORES = 8
DSH = N // NCORES
NGROUPS = DSH // 128
PI = math.pi
TWO_PI = 2.0 * math.pi

# offload a slice of each stage's add pass to the GpSimd engine
GP_OFFLOAD = True


# ===================== custom DVE ops =====================

import concourse.dve_ops as dve_ops
from concourse.dve_spec import Spec, Src0, Src1
from concourse.dve_uop import (
    AluInp,
    AluOp,
    DelayInp,
    DveOpSpec,
    InpSel,
    OutPath,
    OutSel,
    Trigger,
    UopConfig,
)

D = [
    AluInp.PREV_DELAY_0,
    AluInp.PREV_DELAY_1,
    AluInp.PREV_DELAY_2,
    AluInp.PREV_DELAY_3,
    AluInp.PREV_DELAY_4,
    AluInp.PREV_DELAY_5,
]


def _uop(inputs, req0, req1, trigger, next_uop, repeat=0):
    u = UopConfig()
    for lane, sel in enumerate(inputs, start=1):
        u.enable_input(sel, lane)
    u.require_inp0 = req0
    u.require_inp1 = req1
    u.trigger = trigger
    u.next_uop = next_uop
    u.repeat_count = repeat
    return u


_1STATE = dict(
    trigger=(Trigger.SRC_TENSOR_DONE, Trigger.NONE, Trigger.NONE),
    next_uop=(0, 0, 0),
)


# ---------------- CMUL (packed complex multiply, proven) ----------------


def _cmul_uop():
    u = _uop(
        [InpSel.SRC_0, InpSel.SRC_1, InpSel.SRC_0_HI, InpSel.SRC_1_HI],
        1,
        1,
        **_1STATE,
    )
    dp = u.datapath_config
    dp[0].enable_alu(AluOp.MULTIPLY, D[0], D[1])
    dp[0].pass_through_delay(0, 1, 2, 3)
    dp[1].enable_alu(AluOp.MULTIPLY, D[2], D[3])
    dp[1].pass_through_delay(0, 1, 2, 3)
    dp[1].enable_delay_from_src(DelayInp.PREV_ALU_OUT, 4)
    dp[2].enable_alu(AluOp.SUBTRACT, D[4], AluInp.PREV_ALU_OUT)
    dp[2].pass_through_delay(0, 1, 2, 3)
    dp[3].enable_alu(AluOp.MULTIPLY, D[0], D[3])
    dp[3].pass_through_delay(1, 2)
    dp[3].enable_delay_from_src(DelayInp.PREV_ALU_OUT, 4)
    dp[4].enable_alu(AluOp.MULTIPLY, D[2], D[1])
    dp[4].pass_through_delay(4)
    dp[4].enable_delay_from_src(DelayInp.PREV_ALU_OUT, 0)
    dp[5].enable_alu(AluOp.ADD, D[0], AluInp.PREV_ALU_OUT)
    dp[5].pass_through_delay(4)
    dp[6].pass_through_alu()
    dp[6].pass_through_delay(4)
    dp[7].pass_through_alu()
    dp[7].pass_through_delay(4)
    u.enable_output(OutSel.DELAY_4, OutPath.WR0_LO)
    u.enable_output(OutSel.ALU_OUT, OutPath.WR0_HI)
    return u


def _cmul_reference(in0, in1, c0, c1, c2):
    a = in0.astype(np.float32)
    b = np.broadcast_to(in1, in0.shape).astype(np.float32)
    out = np.empty_like(a)
    ar, ai = a[..., 0::2], a[..., 1::2]
    br, bi = b[..., 0::2], b[..., 1::2]
    out[..., 0::2] = ar * br - ai * bi
    out[..., 1::2] = ar * bi + ai * br
    return out


# ---------------- stage-1+2 fused quad ops ----------------
# Each quad of 4 consecutive (pre-permuted) rows (a, b, c, d) produces the
# complex stage-2 outputs written straight into the packed (re, im) layout:
#   y0 = (a+b)+(c+d)           im 0        -> word 4q+0
#   y1 = ((a-b)+C0*(c-d),  C1*(c-d))       -> word 4q+1
#   y2 = (a+b)-(c+d)           im 0        -> word 4q+2
#   y3 = ((a-b)-C0*(c-d), -C1*(c-d))       -> word 4q+3  (C1 passed negated)
# src0 = (a,b) even words of the real plane, src1 = (c,d) odd words.


def _q02_uop(sub: bool):
    u = _uop(
        [InpSel.SRC_0, InpSel.SRC_0_HI, InpSel.SRC_1, InpSel.SRC_1_HI, InpSel.ZERO],
        1,
        1,
        **_1STATE,
    )
    dp = u.datapath_config
    dp[0].enable_alu(AluOp.ADD, D[0], D[1])  # t0 = a+b
    dp[0].pass_through_delay(2, 3, 4)
    dp[1].enable_alu(AluOp.ADD, D[2], D[3])  # t1 = c+d
    dp[1].enable_delay_from_src(DelayInp.PREV_ALU_OUT, 0)  # t0
    dp[1].pass_through_delay(4)
    if sub:
        dp[2].enable_alu(AluOp.SUBTRACT, D[0], AluInp.PREV_ALU_OUT)  # t0-t1
    else:
        dp[2].enable_alu(AluOp.ADD, AluInp.PREV_ALU_OUT, D[0])  # t0+t1
    dp[2].pass_through_delay(4)
    for k in (3, 4, 5, 6, 7):
        dp[k].pass_through_alu()
        dp[k].pass_through_delay(4)
    u.enable_output(OutSel.ALU_OUT, OutPath.WR0_LO)
    u.enable_output(OutSel.DELAY_4, OutPath.WR0_HI)  # zero im
    return u


def _q13_uop(sub: bool):
    u = _uop(
        [
            InpSel.SRC_0,
            InpSel.SRC_0_HI,
            InpSel.SRC_1,
            InpSel.SRC_1_HI,
            InpSel.CONST_0,
            InpSel.ZERO,
        ],
        1,
        1,
        **_1STATE,
    )
    dp = u.datapath_config
    # chains: 0=a 1=b 2=c->p 3=d 4=C0 5=zero
    dp[0].enable_alu(AluOp.SUBTRACT, D[2], D[3])  # u = c-d
    dp[0].pass_through_delay(0, 1, 4, 5)
    dp[1].enable_alu(AluOp.MULTIPLY, AluInp.PREV_ALU_OUT, D[4])  # p = C0*u
    dp[1].pass_through_delay(0, 1, 5)
    dp[2].enable_alu(AluOp.SUBTRACT, D[0], D[1])  # t2 = a-b
    dp[2].enable_delay_from_src(DelayInp.PREV_ALU_OUT, 2)  # p
    dp[2].pass_through_delay(5)
    if sub:
        dp[3].enable_alu(AluOp.SUBTRACT, AluInp.PREV_ALU_OUT, D[2])  # t2 - p
    else:
        dp[3].enable_alu(AluOp.ADD, AluInp.PREV_ALU_OUT, D[2])  # t2 + p
    dp[3].pass_through_delay(5)
    for k in (4, 5, 6, 7):
        dp[k].pass_through_alu()
        dp[k].pass_through_delay(5)
    u.enable_output(OutSel.ALU_OUT, OutPath.WR0_LO)  # y re
    u.enable_output(OutSel.DELAY_5, OutPath.WR0_HI)  # zero im
    return u


# compact im pairs (q, -q), q = C0*(c-d); ACT scatters into the im slots
def _qim_uop():
    u = _uop(
        [
            InpSel.SRC_0,
            InpSel.SRC_0_HI,
            InpSel.SRC_1,
            InpSel.SRC_1_HI,
            InpSel.CONST_0,
            InpSel.ZERO,
        ],
        1,
        1,
        **_1STATE,
    )
    dp = u.datapath_config
    dp[0].enable_alu(AluOp.SUBTRACT, D[2], D[3])  # u = c-d
    dp[0].pass_through_delay(4, 5)
    dp[1].enable_alu(AluOp.MULTIPLY, AluInp.PREV_ALU_OUT, D[4])  # q
    dp[1].pass_through_delay(5)
    dp[2].enable_alu(AluOp.SUBTRACT, D[5], AluInp.PREV_ALU_OUT)  # -q
    dp[2].enable_delay_from_src(DelayInp.PREV_ALU_OUT, 0)  # q
    for k in (3, 4, 5, 6, 7):
        dp[k].pass_through_alu()
        dp[k].pass_through_delay(0)
    u.enable_output(OutSel.DELAY_0, OutPath.WR0_LO)  # q
    u.enable_output(OutSel.ALU_OUT, OutPath.WR0_HI)  # -q
    return u


def _qim_reference(in0, in1, c0, c1, c2):
    b = np.asarray(in1).astype(np.float32)
    ss = np.asarray(c0, np.float32).reshape(-1, *([1] * (b.ndim - 1)))
    q = ss * (b[..., 0::2] - b[..., 1::2])
    out = np.empty_like(b)
    out[..., 0::2] = q
    out[..., 1::2] = -q
    return out


def _q0_reference(in0, in1, c0, c1, c2):
    a = in0.astype(np.float32)
    b = np.asarray(in1).astype(np.float32)
    out = np.empty_like(a)
    out[..., 0::2] = (a[..., 0::2] + a[..., 1::2]) + (b[..., 0::2] + b[..., 1::2])
    out[..., 1::2] = 0.0
    return out


def _q2_reference(in0, in1, c0, c1, c2):
    a = in0.astype(np.float32)
    b = np.asarray(in1).astype(np.float32)
    out = np.empty_like(a)
    out[..., 0::2] = (a[..., 0::2] + a[..., 1::2]) - (b[..., 0::2] + b[..., 1::2])
    out[..., 1::2] = 0.0
    return out


def _q13_reference(sub):
    def ref(in0, in1, c0, c1, c2):
        a = in0.astype(np.float32)
        b = np.asarray(in1).astype(np.float32)
        cc = np.asarray(c0, np.float32).reshape(-1, *([1] * (a.ndim - 1)))
        t2 = a[..., 0::2] - a[..., 1::2]
        u = b[..., 0::2] - b[..., 1::2]
        out = np.empty_like(a)
        out[..., 0::2] = t2 - cc * u if sub else t2 + cc * u
        out[..., 1::2] = 0.0
        return out

    return ref


# ---------------- registry ----------------


class RawDveOp:
    def __init__(self, name, mk_all, rd1_en, perf_max, reference):
        self.name = name
        self.subdim = False
        self.spec = Spec(body=Src0 * Src1 if rd1_en else Src0, reference=reference)
        self.rd1_en = rd1_en
        self.perf_max = perf_max
        self._mk = mk_all
        self._cache = {}

    def compile(self, ver):
        if ver in self._cache:
            return self._cache[ver]
        kw = self._mk()
        spec = DveOpSpec(
            name=self.name,
            opcode=dve_ops.get_dve_sub_opcode(self.name),
            perf_max=self.perf_max,
            rd1_en=self.rd1_en,
            **kw,
        )
        spec.validate(ver)
        self._cache[ver] = spec
        return spec


RAW_OPS = {}


def register_raw_ops():
    if RAW_OPS:
        return RAW_OPS
    defs = [
        RawDveOp(
            "CMUL_PACKED_ANT",
            lambda: dict(uops=[_cmul_uop()], uops_2x=[_cmul_uop()]),
            True,
            1,
            _cmul_reference,
        ),
        RawDveOp(
            "QUAD0_ANT",
            lambda: dict(uops=[_q02_uop(False)], uops_2x=[_q02_uop(False)]),
            True,
            1,
            _q0_reference,
        ),
        RawDveOp(
            "QUAD2_ANT",
            lambda: dict(uops=[_q02_uop(True)], uops_2x=[_q02_uop(True)]),
            True,
            1,
            _q2_reference,
        ),
        RawDveOp(
            "QUAD1_ANT",
            lambda: dict(uops=[_q13_uop(False)], uops_2x=[_q13_uop(False)]),
            True,
            1,
            _q13_reference(False),
        ),
        RawDveOp(
            "QUAD3_ANT",
            lambda: dict(uops=[_q13_uop(True)], uops_2x=[_q13_uop(True)]),
            True,
            1,
            _q13_reference(True),
        ),
        RawDveOp(
            "QIM_ANT",
            lambda: dict(uops=[_qim_uop()], uops_2x=[_qim_uop()]),
            True,
            1,
            _qim_reference,
        ),
    ]
    for op in defs:
        if op.name not in dve_ops._SUB_OPCODE_FOR_NAME:
            dve_ops.OPS.append(op)
            row = dve_ops._CUSTOM_DVE_ROW_BASE + len(dve_ops.OPS) - 1
            assert row < 0x20
            dve_ops._SUB_OPCODE_FOR_NAME[op.name] = row
            dve_ops.CUSTOM_DVE_SPECS[op.name] = op.spec
        RAW_OPS[op.name] = op
    return RAW_OPS


def emit_raw(nc, name, out, in0, in1=None, s0=None, s1=None):
    import concourse.bass_isa as bass_isa

    ops = register_raw_ops()
    op = ops[name]
    v = nc.vector
    if op.name not in nc.m.ant_custom_dve_ops:
        nc.m.ant_custom_dve_ops = sorted({*nc.m.ant_custom_dve_ops, op.name})
    shape = (
        bass_isa.CustomDveShape.STT
        if in1 is not None
        else bass_isa.CustomDveShape.TTSS
    )
    isa_opcode = nc.isa.Opcode[
        f"NEURON_ISA_TPB_OPCODE_CUSTOM_DVE_ANT_{shape.slot()}"
    ].value
    imm = mybir.ImmediateValue(dtype=mybir.dt.float32, value=0.0)
    s0a = v.lower_ap(s0, for_isa=True) if s0 is not None else imm
    s1a = v.lower_ap(s1, for_isa=True) if s1 is not None else imm
    ins = [v.lower_ap(in0, for_isa=True)]
    if in1 is not None:
        ins.append(v.lower_ap(in1, for_isa=True))
    ins += [s0a, s1a]
    return v.add_instruction(
        bass_isa.InstCustomDveAnt(
            name=nc.get_next_instruction_name(),
            op_name=op.name,
            rd1_en=op.rd1_en,
            subdim=0,
            imm2=0.0,
            shape=shape,
            row=dve_ops.get_dve_sub_opcode(op.name),
            isa_opcode=isa_opcode,
            ins=ins,
            outs=[v.lower_ap(out, for_isa=True)],
        )
    )


def patch_perf_bits(nc):
    ops = register_raw_ops()
    n = 0
    for fn in nc.m.functions:
        for blk in fn.blocks:
            for inst in blk.instructions:
                nm = getattr(inst, "op_name", None)
                if nm in ops:
                    bb = bytearray(bytes(inst.instr))
                    bb[36] |= ops[nm].perf_max << 6
                    inst.instr = bytes(bb)
                    n += 1
    return n


# ===================== kernel builder =====================


def build_fft_nc():
    register_raw_ops()
    nc = bacc.Bacc()

    xT = nc.dram_tensor("xT", [DSH, N], F16, kind="ExternalInput")
    wT = nc.dram_tensor("wT", [DSH, N // 2], F32, kind="ExternalInput")
    wc = nc.dram_tensor("wc", [DSH, 1], F32, kind="ExternalInput")
    outT = nc.dram_tensor("outT", [DSH, 2 * N], F16, kind="ExternalOutput")

    # const AP: pi/2 bias for the cos path
    HPI = float(np.float32(PI / 2))
    chp = nc.alloc_sbuf_tensor("const-f32-hpi", [128, 1], F32)
    nc.gpsimd.memset(chp.ap(), HPI)
    nc.const_aps.aps[(F32, HPI)] = chp.ap()
    nc.all_engine_barrier()

    with TileContext(nc) as tc:
        with (
            tc.tile_pool(name="xr", bufs=2) as xrpool,
            tc.tile_pool(name="xbuf", bufs=2) as xpool,
            tc.tile_pool(name="tmp", bufs=2) as tpool,
            tc.tile_pool(name="tw", bufs=2) as twpool,
            tc.tile_pool(name="ph", bufs=2) as ppool,
            tc.tile_pool(name="col", bufs=2) as colpool,
        ):
            for g in range(NGROUPS):
                rows = slice(g * 128, (g + 1) * 128)

                # ---- tiny stage-2 column phases first (unblocks quads) ----
                rc = colpool.tile([128, 1], F32, tag="rc")
                nc.sync.dma_start(rc[:], wc[rows, :])
                cols = colpool.tile([128, 3], F32, tag="cols")
                nc.scalar.activation(cols[:, 2:3], rc[:], AF.Abs)
                nc.scalar.activation(
                    cols[:, 0:1], cols[:, 2:3], AF.Sin, scale=-TWO_PI, bias=HPI
                )  # c = cos
                nc.scalar.activation(cols[:, 1:2], rc[:], AF.Sin, scale=TWO_PI)  # s

                # ---- x real plane (host pre-permuted fp16) ----
                xr = xrpool.tile([128, N], F16, tag="xplane")
                nc.sync.dma_start(xr[:], xT[rows, :])

                # ---- stages 1+2: four fused quad ops -> packed complex x ----
                x = xpool.tile([128, 2 * N], F16, tag="x")
                xr4 = xr[:].rearrange("p (b f) -> p b f", f=4)
                src0 = xr4[:, :, 0:2]
                src1 = xr4[:, :, 2:4]
                x8 = x[:].rearrange("p (b f) -> p b f", f=8)
                emit_raw(nc, "QUAD0_ANT", x8[:, :, 0:2], src0, src1)
                emit_raw(
                    nc, "QUAD1_ANT", x8[:, :, 2:4], src0, src1, s0=cols[:, 0:1]
                )
                emit_raw(nc, "QUAD2_ANT", x8[:, :, 4:6], src0, src1)
                emit_raw(
                    nc, "QUAD3_ANT", x8[:, :, 6:8], src0, src1, s0=cols[:, 0:1]
                )
                # im parts of rows 4q+1 / 4q+3: compact (q, -q) pairs on the
                # Vector engine, scattered into the packed im slots by ACT
                imc = tpool.tile([128, N // 2], F16, tag="imc")
                imc2 = imc[:].rearrange("p (b f) -> p b f", f=2)
                emit_raw(nc, "QIM_ANT", imc2, src0, src1, s0=cols[:, 1:2])
                nc.scalar.activation(x8[:, :, 3:8:4], imc2, AF.Copy)

                # ---- phases arrive host-reduced: wT[p,k] = r_red in
                # [-0.5, 0.5] with sin(2pi*r_red) = sin(phi) ----
                r = ppool.tile([128, N // 2], F32, tag="r")
                nc.sync.dma_start(r[:], wT[rows, :])
                absr = ppool.tile([128, N // 2], F32, tag="absr")
                nc.scalar.activation(absr[:], r[:], AF.Abs)

                # ---- interleaved twiddle packs: stage s at [2*half, 4*half) ----
                pack = twpool.tile([128, 2 * N], F16, tag="pack")
                for s in range(3, LOGN + 1):
                    half = 1 << (s - 1)
                    stride = N >> s
                    src_im = r[:, 0 : N // 2 : stride]
                    src_re = absr[:, 0 : N // 2 : stride]
                    nc.scalar.activation(
                        pack[:, 2 * half : 4 * half : 2],
                        src_re,
                        AF.Sin,
                        scale=-TWO_PI,
                        bias=HPI,
                    )
                    nc.scalar.activation(
                        pack[:, 2 * half + 1 : 4 * half : 2],
                        src_im,
                        AF.Sin,
                        scale=TWO_PI,
                    )

                t1 = tpool.tile([128, N], F16, tag="t1")  # packed cmul temp

                # ---- stages 3..11: packed generic. A 512-word slice of the
                # add pass runs on GpSimd: even-index blocks in the first
                # half -- next stage's CMUL (odd blocks) never reads them,
                # only the later sub/add do, so GpSimd overlaps fully. ----
                for s in range(3, LOGN):
                    step = 1 << s
                    half = step // 2
                    nb = N // step

                    xv = x[:].rearrange("p (b stc) -> p b stc", stc=2 * step)
                    top = xv[:, :, 0 : 2 * half]
                    bot = xv[:, :, 2 * half : 2 * step]
                    tw = (
                        pack[:, 2 * half : 4 * half]
                        .unsqueeze(1)
                        .broadcast_to([128, nb, 2 * half])
                    )
                    tv = t1[:, 0 : nb * 2 * half]
                    tvb = tv.rearrange("p (b h) -> p b h", h=2 * half)
                    if nb > 1:
                        tv = tvb
                    emit_raw(nc, "CMUL_PACKED_ANT", tv, bot, tw)
                    nc.vector.tensor_sub(bot, top, tv)
                    if not GP_OFFLOAD:
                        nc.vector.tensor_add(top, top, tv)
                    elif nb >= 4:
                        h2 = nb // 2
                        nc.gpsimd.tensor_add(
                            xv[:, 0:h2:2, 0 : 2 * half],
                            xv[:, 0:h2:2, 0 : 2 * half],
                            tvb[:, 0:h2:2],
                        )
                        nc.vector.tensor_add(
                            xv[:, 1:h2:2, 0 : 2 * half],
                            xv[:, 1:h2:2, 0 : 2 * half],
                            tvb[:, 1:h2:2],
                        )
                        nc.vector.tensor_add(
                            xv[:, h2:nb, 0 : 2 * half],
                            xv[:, h2:nb, 0 : 2 * half],
                            tvb[:, h2:nb],
                        )
                    else:
                        # s == 11: nb == 2; gpsimd takes the first half of
                        # block 0's top (consumed by stage 12's late ops)
                        nc.gpsimd.tensor_add(
                            xv[:, 0:1, 0:half],
                            xv[:, 0:1, 0:half],
                            tvb[:, 0:1, 0:half],
                        )
                        nc.vector.tensor_add(
                            xv[:, 0:1, half : 2 * half],
                            xv[:, 0:1, half : 2 * half],
                            tvb[:, 0:1, half : 2 * half],
                        )
                        nc.vector.tensor_add(
                            xv[:, 1:2, 0 : 2 * half],
                            xv[:, 1:2, 0 : 2 * half],
                            tvb[:, 1:2],
                        )

                # ---- stage 12 in two column chunks; outputs DMA straight to
                # HBM as packed fp16 ----
                Q = N // 2
                for c in range(2):
                    top_c = x[:, c * Q : c * Q + Q]
                    bot_c = x[:, N + c * Q : N + c * Q + Q]
                    tw_c = pack[:, N + c * Q : N + c * Q + Q]
                    tv = t1[:, 0:Q]
                    emit_raw(nc, "CMUL_PACKED_ANT", tv, bot_c, tw_c)
                    nc.vector.tensor_sub(bot_c, top_c, tv)
                    hq = 2 + c
                    nc.sync.dma_start(
                        outT[rows, hq * Q : (hq + 1) * Q], x[:, hq * Q : (hq + 1) * Q]
                    )
                    nc.vector.tensor_add(top_c, top_c, tv)
                    nc.sync.dma_start(
                        outT[rows, c * Q : (c + 1) * Q], x[:, c * Q : (c + 1) * Q]
                    )

    nc.compile()
    patch_perf_bits(nc)
    return nc


# ===================== host glue =====================

_PERM = None


def _perm():
    global _PERM
    if _PERM is None:
        _PERM = np.arange(N) ^ (N // 2)
    return _PERM


def make_core_inputs(x: np.ndarray, weights: np.ndarray, core: int):
    sl = slice(core * DSH, (core + 1) * DSH)
    xp = x[_perm()][:, sl]  # pre-permuted rows
    xT = np.ascontiguousarray(xp.T).astype(np.float16)
    w = weights[: N // 2, sl].astype(np.float64)
    k = -(1.0 / N) * np.arange(N // 2, dtype=np.float64)
    rr = w * k[:, None]
    rr -= np.rint(rr)
    wT = np.ascontiguousarray(rr.T).astype(np.float32)
    wc = np.ascontiguousarray(wT[:, 1024:1025])
    return {"xT": xT, "wT": wT, "wc": wc}


def assemble_output(core_outs):
    full = np.empty((N, N), dtype=np.complex64)
    for c, r in enumerate(core_outs):
        oc = r["outT"].astype(np.float32).view(np.complex64)
        full[:, c * DSH : (c + 1) * DSH] = oc.T
    return full


_NC_CACHE = None


def get_nc():
    global _NC_CACHE
    if _NC_CACHE is None:
        _NC_CACHE = build_fft_nc()
    return _NC_CACHE


def make_in_maps(x: np.ndarray, weights: np.ndarray):
    x = np.asarray(x, dtype=np.float32)
    weights = np.asarray(weights, dtype=np.float32)
    in_maps = [make_core_inputs(x, weights, c) for c in range(NCORES)]
    return in_maps


def run_on_hw(x, weights, **spmd_kwargs):
    nc = get_nc()
    in_maps = make_in_maps(x, weights)
    res = run_bass_kernel_spmd(nc, in_maps, core_ids=list(range(NCORES)), **spmd_kwargs)
    return assemble_output(res.results), res


def kernel(x: np.ndarray, weights: np.ndarray) -> np.ndarray:
    out, _ = run_on_hw(x, weights)
    return out


# revision 10
# speedup vs baseline: 1.3407x; 1.3407x over previous
"""Trainium2 Bass kernel: data-dependent radix-2 FFT butterfly network.

out = FFT-like transform of x (4096x4096 f32 -> complex64); stage twiddles
are exp(-2j*pi*k/N * weights[k, :]) (learned, per-feature), N = 4096,
12 radix-2 stages, initial row permutation j ^ N/2.

Sharding: feature dim split across 8 NeuronCores (512 each) - the whole
network is elementwise along features, so no cross-core communication.

Per-core: features on partitions (4 groups of 128), FFT rows along the
free dim, x stored as packed (re, im) fp16 pairs. Each generic stage is
3 Vector-engine ops: a packed-complex-multiply custom DVE op (one
complex/cycle in 2X_1PORT mode) plus packed fp16 add/subs in 2x mode;
part of the butterfly add/sub work is offloaded to the otherwise-idle
GpSimd engine each stage. Stages 1-2 (real inputs, trivial twiddles) are
four fused quad ops that write the packed complex layout directly.
Twiddles are generated on-device by the Scalar engine's Sin LUT from
host-range-reduced phases. I/O is fp16 end-to-end: the host pre-permutes
(j ^ N/2), transposes and converts x to fp16, and the packed fp16 output
is converted to complex64 on the host. All tile pools are double-
buffered so consecutive 128-feature groups overlap.
"""

import math
import sys

import numpy as np

if "/opt/trn_rl_repo" not in sys.path:
    sys.path.insert(0, "/opt/trn_rl_repo")

import concourse.bacc as bacc
import concourse.bass as bass
import concourse.mybir as mybir
from concourse.bass_utils import run_bass_kernel_spmd
from concourse.tile import TileContext

F32 = mybir.dt.float32
F16 = mybir.dt.float16
AF = mybir.ActivationFunctionType
ALU = mybir.AluOpType

N = 4096
LOGN = 12
NCORES = 8
DSH = N // NCORES
NGROUPS = DSH // 128
PI = math.pi
TWO_PI = 2.0 * math.pi

# offload a slice of each stage's add pass to the GpSimd engine
# (measured net-negative: GpSimd's SBUF port contends with the DVE and slows
# every Vector op ~30%; keep off)
GP_OFFLOAD = False


# ===================== custom DVE ops =====================

import concourse.dve_ops as dve_ops
from concourse.dve_spec import Spec, Src0, Src1
from concourse.dve_uop import (
    AluInp,
    AluOp,
    DelayInp,
    DveOpSpec,
    InpSel,
    OutPath,
    OutSel,
    Trigger,
    UopConfig,
)

D = [
    AluInp.PREV_DELAY_0,
    AluInp.PREV_DELAY_1,
    AluInp.PREV_DELAY_2,
    AluInp.PREV_DELAY_3,
    AluInp.PREV_DELAY_4,
    AluInp.PREV_DELAY_5,
]


def _uop(inputs, req0, req1, trigger, next_uop, repeat=0):
    u = UopConfig()
    for lane, sel in enumerate(inputs, start=1):
        u.enable_input(sel, lane)
    u.require_inp0 = req0
    u.require_inp1 = req1
    u.trigger = trigger
    u.next_uop = next_uop
    u.repeat_count = repeat
    return u


_1STATE = dict(
    trigger=(Trigger.SRC_TENSOR_DONE, Trigger.NONE, Trigger.NONE),
    next_uop=(0, 0, 0),
)


# ---------------- CMUL (packed complex multiply, proven) ----------------


def _cmul_uop():
    u = _uop(
        [InpSel.SRC_0, InpSel.SRC_1, InpSel.SRC_0_HI, InpSel.SRC_1_HI],
        1,
        1,
        **_1STATE,
    )
    dp = u.datapath_config
    dp[0].enable_alu(AluOp.MULTIPLY, D[0], D[1])
    dp[0].pass_through_delay(0, 1, 2, 3)
    dp[1].enable_alu(AluOp.MULTIPLY, D[2], D[3])
    dp[1].pass_through_delay(0, 1, 2, 3)
    dp[1].enable_delay_from_src(DelayInp.PREV_ALU_OUT, 4)
    dp[2].enable_alu(AluOp.SUBTRACT, D[4], AluInp.PREV_ALU_OUT)
    dp[2].pass_through_delay(0, 1, 2, 3)
    dp[3].enable_alu(AluOp.MULTIPLY, D[0], D[3])
    dp[3].pass_through_delay(1, 2)
    dp[3].enable_delay_from_src(DelayInp.PREV_ALU_OUT, 4)
    dp[4].enable_alu(AluOp.MULTIPLY, D[2], D[1])
    dp[4].pass_through_delay(4)
    dp[4].enable_delay_from_src(DelayInp.PREV_ALU_OUT, 0)
    dp[5].enable_alu(AluOp.ADD, D[0], AluInp.PREV_ALU_OUT)
    dp[5].pass_through_delay(4)
    dp[6].pass_through_alu()
    dp[6].pass_through_delay(4)
    dp[7].pass_through_alu()
    dp[7].pass_through_delay(4)
    u.enable_output(OutSel.DELAY_4, OutPath.WR0_LO)
    u.enable_output(OutSel.ALU_OUT, OutPath.WR0_HI)
    return u


def _cmul_reference(in0, in1, c0, c1, c2):
    a = in0.astype(np.float32)
    b = np.broadcast_to(in1, in0.shape).astype(np.float32)
    out = np.empty_like(a)
    ar, ai = a[..., 0::2], a[..., 1::2]
    br, bi = b[..., 0::2], b[..., 1::2]
    out[..., 0::2] = ar * br - ai * bi
    out[..., 1::2] = ar * bi + ai * br
    return out


# ---------------- stage-1+2 fused quad ops ----------------
# Each quad of 4 consecutive (pre-permuted) rows (a, b, c, d) produces the
# complex stage-2 outputs written straight into the packed (re, im) layout:
#   y0 = (a+b)+(c+d)           im 0        -> word 4q+0
#   y1 = ((a-b)+C0*(c-d),  C1*(c-d))       -> word 4q+1
#   y2 = (a+b)-(c+d)           im 0        -> word 4q+2
#   y3 = ((a-b)-C0*(c-d), -C1*(c-d))       -> word 4q+3  (C1 passed negated)
# src0 = (a,b) even words of the real plane, src1 = (c,d) odd words.


def _q02_uop(sub: bool):
    u = _uop(
        [InpSel.SRC_0, InpSel.SRC_0_HI, InpSel.SRC_1, InpSel.SRC_1_HI, InpSel.ZERO],
        1,
        1,
        **_1STATE,
    )
    dp = u.datapath_config
    dp[0].enable_alu(AluOp.ADD, D[0], D[1])  # t0 = a+b
    dp[0].pass_through_delay(2, 3, 4)
    dp[1].enable_alu(AluOp.ADD, D[2], D[3])  # t1 = c+d
    dp[1].enable_delay_from_src(DelayInp.PREV_ALU_OUT, 0)  # t0
    dp[1].pass_through_delay(4)
    if sub:
        dp[2].enable_alu(AluOp.SUBTRACT, D[0], AluInp.PREV_ALU_OUT)  # t0-t1
    else:
        dp[2].enable_alu(AluOp.ADD, AluInp.PREV_ALU_OUT, D[0])  # t0+t1
    dp[2].pass_through_delay(4)
    for k in (3, 4, 5, 6, 7):
        dp[k].pass_through_alu()
        dp[k].pass_through_delay(4)
    u.enable_output(OutSel.ALU_OUT, OutPath.WR0_LO)
    u.enable_output(OutSel.DELAY_4, OutPath.WR0_HI)  # zero im
    return u


def _q13_uop(sub: bool):
    u = _uop(
        [
            InpSel.SRC_0,
            InpSel.SRC_0_HI,
            InpSel.SRC_1,
            InpSel.SRC_1_HI,
            InpSel.CONST_0,
            InpSel.ZERO,
        ],
        1,
        1,
        **_1STATE,
    )
    dp = u.datapath_config
    # chains: 0=a 1=b 2=c->p 3=d 4=C0 5=zero
    dp[0].enable_alu(AluOp.SUBTRACT, D[2], D[3])  # u = c-d
    dp[0].pass_through_delay(0, 1, 4, 5)
    dp[1].enable_alu(AluOp.MULTIPLY, AluInp.PREV_ALU_OUT, D[4])  # p = C0*u
    dp[1].pass_through_delay(0, 1, 5)
    dp[2].enable_alu(AluOp.SUBTRACT, D[0], D[1])  # t2 = a-b
    dp[2].enable_delay_from_src(DelayInp.PREV_ALU_OUT, 2)  # p
    dp[2].pass_through_delay(5)
    if sub:
        dp[3].enable_alu(AluOp.SUBTRACT, AluInp.PREV_ALU_OUT, D[2])  # t2 - p
    else:
        dp[3].enable_alu(AluOp.ADD, AluInp.PREV_ALU_OUT, D[2])  # t2 + p
    dp[3].pass_through_delay(5)
    for k in (4, 5, 6, 7):
        dp[k].pass_through_alu()
        dp[k].pass_through_delay(5)
    u.enable_output(OutSel.ALU_OUT, OutPath.WR0_LO)  # y re
    u.enable_output(OutSel.DELAY_5, OutPath.WR0_HI)  # zero im
    return u


# compact im pairs (q, -q), q = C0*(c-d); ACT scatters into the im slots
def _qim_uop():
    u = _uop(
        [
            InpSel.SRC_0,
            InpSel.SRC_0_HI,
            InpSel.SRC_1,
            InpSel.SRC_1_HI,
            InpSel.CONST_0,
            InpSel.ZERO,
        ],
        1,
        1,
        **_1STATE,
    )
    dp = u.datapath_config
    dp[0].enable_alu(AluOp.SUBTRACT, D[2], D[3])  # u = c-d
    dp[0].pass_through_delay(4, 5)
    dp[1].enable_alu(AluOp.MULTIPLY, AluInp.PREV_ALU_OUT, D[4])  # q
    dp[1].pass_through_delay(5)
    dp[2].enable_alu(AluOp.SUBTRACT, D[5], AluInp.PREV_ALU_OUT)  # -q
    dp[2].enable_delay_from_src(DelayInp.PREV_ALU_OUT, 0)  # q
    for k in (3, 4, 5, 6, 7):
        dp[k].pass_through_alu()
        dp[k].pass_through_delay(0)
    u.enable_output(OutSel.DELAY_0, OutPath.WR0_LO)  # q
    u.enable_output(OutSel.ALU_OUT, OutPath.WR0_HI)  # -q
    return u


def _qim_reference(in0, in1, c0, c1, c2):
    b = np.asarray(in1).astype(np.float32)
    ss = np.asarray(c0, np.float32).reshape(-1, *([1] * (b.ndim - 1)))
    q = ss * (b[..., 0::2] - b[..., 1::2])
    out = np.empty_like(b)
    out[..., 0::2] = q
    out[..., 1::2] = -q
    return out


def _q0_reference(in0, in1, c0, c1, c2):
    a = in0.astype(np.float32)
    b = np.asarray(in1).astype(np.float32)
    out = np.empty_like(a)
    out[..., 0::2] = (a[..., 0::2] + a[..., 1::2]) + (b[..., 0::2] + b[..., 1::2])
    out[..., 1::2] = 0.0
    return out


def _q2_reference(in0, in1, c0, c1, c2):
    a = in0.astype(np.float32)
    b = np.asarray(in1).astype(np.float32)
    out = np.empty_like(a)
    out[..., 0::2] = (a[..., 0::2] + a[..., 1::2]) - (b[..., 0::2] + b[..., 1::2])
    out[..., 1::2] = 0.0
    return out


def _q13_reference(sub):
    def ref(in0, in1, c0, c1, c2):
        a = in0.astype(np.float32)
        b = np.asarray(in1).astype(np.float32)
        cc = np.asarray(c0, np.float32).reshape(-1, *([1] * (a.ndim - 1)))
        t2 = a[..., 0::2] - a[..., 1::2]
        u = b[..., 0::2] - b[..., 1::2]
        out = np.empty_like(a)
        out[..., 0::2] = t2 - cc * u if sub else t2 + cc * u
        out[..., 1::2] = 0.0
        return out

    return ref


# ---------------- registry ----------------


class RawDveOp:
    def __init__(self, name, mk_all, rd1_en, perf_max, reference):
        self.name = name
        self.subdim = False
        self.spec = Spec(body=Src0 * Src1 if rd1_en else Src0, reference=reference)
        self.rd1_en = rd1_en
        self.perf_max = perf_max
        self._mk = mk_all
        self._cache = {}

    def compile(self, ver):
        if ver in self._cache:
            return self._cache[ver]
        kw = self._mk()
        spec = DveOpSpec(
            name=self.name,
            opcode=dve_ops.get_dve_sub_opcode(self.name),
            perf_max=self.perf_max,
            rd1_en=self.rd1_en,
            **kw,
        )
        spec.validate(ver)
        self._cache[ver] = spec
        return spec


RAW_OPS = {}


def register_raw_ops():
    if RAW_OPS:
        return RAW_OPS
    defs = [
        RawDveOp(
            "CMUL_PACKED_ANT",
            lambda: dict(uops=[_cmul_uop()], uops_2x=[_cmul_uop()]),
            True,
            1,
            _cmul_reference,
        ),
        RawDveOp(
            "QUAD0_ANT",
            lambda: dict(uops=[_q02_uop(False)], uops_2x=[_q02_uop(False)]),
            True,
            1,
            _q0_reference,
        ),
        RawDveOp(
            "QUAD2_ANT",
            lambda: dict(uops=[_q02_uop(True)], uops_2x=[_q02_uop(True)]),
            True,
            1,
            _q2_reference,
        ),
        RawDveOp(
            "QUAD1_ANT",
            lambda: dict(uops=[_q13_uop(False)], uops_2x=[_q13_uop(False)]),
            True,
            1,
            _q13_reference(False),
        ),
        RawDveOp(
            "QUAD3_ANT",
            lambda: dict(uops=[_q13_uop(True)], uops_2x=[_q13_uop(True)]),
            True,
            1,
            _q13_reference(True),
        ),
        RawDveOp(
            "QIM_ANT",
            lambda: dict(uops=[_qim_uop()], uops_2x=[_qim_uop()]),
            True,
            1,
            _qim_reference,
        ),
    ]
    for op in defs:
        if op.name not in dve_ops._SUB_OPCODE_FOR_NAME:
            dve_ops.OPS.append(op)
            row = dve_ops._CUSTOM_DVE_ROW_BASE + len(dve_ops.OPS) - 1
            assert row < 0x20
            dve_ops._SUB_OPCODE_FOR_NAME[op.name] = row
            dve_ops.CUSTOM_DVE_SPECS[op.name] = op.spec
        RAW_OPS[op.name] = op
    return RAW_OPS


def emit_raw(nc, name, out, in0, in1=None, s0=None, s1=None):
    import concourse.bass_isa as bass_isa

    ops = register_raw_ops()
    op = ops[name]
    v = nc.vector
    if op.name not in nc.m.ant_custom_dve_ops:
        nc.m.ant_custom_dve_ops = sorted({*nc.m.ant_custom_dve_ops, op.name})
    shape = (
        bass_isa.CustomDveShape.STT
        if in1 is not None
        else bass_isa.CustomDveShape.TTSS
    )
    isa_opcode = nc.isa.Opcode[
        f"NEURON_ISA_TPB_OPCODE_CUSTOM_DVE_ANT_{shape.slot()}"
    ].value
    imm = mybir.ImmediateValue(dtype=mybir.dt.float32, value=0.0)
    s0a = v.lower_ap(s0, for_isa=True) if s0 is not None else imm
    s1a = v.lower_ap(s1, for_isa=True) if s1 is not None else imm
    ins = [v.lower_ap(in0, for_isa=True)]
    if in1 is not None:
        ins.append(v.lower_ap(in1, for_isa=True))
    ins += [s0a, s1a]
    return v.add_instruction(
        bass_isa.InstCustomDveAnt(
            name=nc.get_next_instruction_name(),
            op_name=op.name,
            rd1_en=op.rd1_en,
            subdim=0,
            imm2=0.0,
            shape=shape,
            row=dve_ops.get_dve_sub_opcode(op.name),
            isa_opcode=isa_opcode,
            ins=ins,
            outs=[v.lower_ap(out, for_isa=True)],
        )
    )


def patch_perf_bits(nc):
    ops = register_raw_ops()
    n = 0
    for fn in nc.m.functions:
        for blk in fn.blocks:
            for inst in blk.instructions:
                nm = getattr(inst, "op_name", None)
                if nm in ops:
                    bb = bytearray(bytes(inst.instr))
                    bb[36] |= ops[nm].perf_max << 6
                    inst.instr = bytes(bb)
                    n += 1
    return n


# ===================== kernel builder =====================


def build_fft_nc():
    register_raw_ops()
    nc = bacc.Bacc()

    xT = nc.dram_tensor("xT", [DSH, N], F16, kind="ExternalInput")
    wT = nc.dram_tensor("wT", [DSH, N // 2], F32, kind="ExternalInput")
    wc = nc.dram_tensor("wc", [DSH, 1], F32, kind="ExternalInput")
    outT = nc.dram_tensor("outT", [DSH, 2 * N], F16, kind="ExternalOutput")

    # const AP: pi/2 bias for the cos path
    HPI = float(np.float32(PI / 2))
    chp = nc.alloc_sbuf_tensor("const-f32-hpi", [128, 1], F32)
    nc.gpsimd.memset(chp.ap(), HPI)
    nc.const_aps.aps[(F32, HPI)] = chp.ap()
    nc.all_engine_barrier()

    with TileContext(nc) as tc:
        with (
            tc.tile_pool(name="xr", bufs=2) as xrpool,
            tc.tile_pool(name="xbuf", bufs=2) as xpool,
            tc.tile_pool(name="tmp", bufs=2) as tpool,
            tc.tile_pool(name="tw", bufs=2) as twpool,
            tc.tile_pool(name="ph", bufs=2) as ppool,
            tc.tile_pool(name="col", bufs=2) as colpool,
        ):
            for g in range(NGROUPS):
                rows = slice(g * 128, (g + 1) * 128)

                # ---- tiny stage-2 column phases first (unblocks quads) ----
                rc = colpool.tile([128, 1], F32, tag="rc")
                nc.sync.dma_start(rc[:], wc[rows, :])
                cols = colpool.tile([128, 3], F32, tag="cols")
                nc.scalar.activation(cols[:, 2:3], rc[:], AF.Abs)
                nc.scalar.activation(
                    cols[:, 0:1], cols[:, 2:3], AF.Sin, scale=-TWO_PI, bias=HPI
                )  # c = cos
                nc.scalar.activation(cols[:, 1:2], rc[:], AF.Sin, scale=TWO_PI)  # s

                # ---- x real plane (host pre-permuted fp16) ----
                xr = xrpool.tile([128, N], F16, tag="xplane")
                nc.sync.dma_start(xr[:], xT[rows, :])

                # ---- stages 1+2: four fused quad ops -> packed complex x ----
                x = xpool.tile([128, 2 * N], F16, tag="x")
                xr4 = xr[:].rearrange("p (b f) -> p b f", f=4)
                src0 = xr4[:, :, 0:2]
                src1 = xr4[:, :, 2:4]
                x8 = x[:].rearrange("p (b f) -> p b f", f=8)
                emit_raw(nc, "QUAD0_ANT", x8[:, :, 0:2], src0, src1)
                emit_raw(
                    nc, "QUAD1_ANT", x8[:, :, 2:4], src0, src1, s0=cols[:, 0:1]
                )
                emit_raw(nc, "QUAD2_ANT", x8[:, :, 4:6], src0, src1)
                emit_raw(
                    nc, "QUAD3_ANT", x8[:, :, 6:8], src0, src1, s0=cols[:, 0:1]
                )
                # im parts of rows 4q+1 / 4q+3: compact (q, -q) pairs on the
                # Vector engine, scattered into the packed im slots by ACT
                imc = tpool.tile([128, N // 2], F16, tag="imc")
                imc2 = imc[:].rearrange("p (b f) -> p b f", f=2)
                emit_raw(nc, "QIM_ANT", imc2, src0, src1, s0=cols[:, 1:2])
                nc.scalar.activation(x8[:, :, 3:8:4], imc2, AF.Copy)

                # ---- phases arrive host-reduced: wT[p,k] = r_red in
                # [-0.5, 0.5] with sin(2pi*r_red) = sin(phi) ----
                r = ppool.tile([128, N // 2], F32, tag="r")
                nc.sync.dma_start(r[:], wT[rows, :])
                absr = ppool.tile([128, N // 2], F32, tag="absr")
                nc.scalar.activation(absr[:], r[:], AF.Abs)

                # ---- interleaved twiddle packs: stage s at [2*half, 4*half) ----
                pack = twpool.tile([128, 2 * N], F16, tag="pack")
                for s in range(3, LOGN + 1):
                    half = 1 << (s - 1)
                    stride = N >> s
                    src_im = r[:, 0 : N // 2 : stride]
                    src_re = absr[:, 0 : N // 2 : stride]
                    nc.scalar.activation(
                        pack[:, 2 * half : 4 * half : 2],
                        src_re,
                        AF.Sin,
                        scale=-TWO_PI,
                        bias=HPI,
                    )
                    nc.scalar.activation(
                        pack[:, 2 * half + 1 : 4 * half : 2],
                        src_im,
                        AF.Sin,
                        scale=TWO_PI,
                    )

                t1 = tpool.tile([128, N], F16, tag="t1")  # packed cmul temp

                # ---- stages 3..11: packed generic. A 512-word slice of the
                # add pass runs on GpSimd: even-index blocks in the first
                # half -- next stage's CMUL (odd blocks) never reads them,
                # only the later sub/add do, so GpSimd overlaps fully. ----
                for s in range(3, LOGN):
                    step = 1 << s
                    half = step // 2
                    nb = N // step

                    xv = x[:].rearrange("p (b stc) -> p b stc", stc=2 * step)
                    top = xv[:, :, 0 : 2 * half]
                    bot = xv[:, :, 2 * half : 2 * step]
                    tw = (
                        pack[:, 2 * half : 4 * half]
                        .unsqueeze(1)
                        .broadcast_to([128, nb, 2 * half])
                    )
                    tv = t1[:, 0 : nb * 2 * half]
                    tvb = tv.rearrange("p (b h) -> p b h", h=2 * half)
                    if nb > 1:
                        tv = tvb
                    emit_raw(nc, "CMUL_PACKED_ANT", tv, bot, tw)
                    nc.vector.tensor_sub(bot, top, tv)
                    if not GP_OFFLOAD:
                        nc.vector.tensor_add(top, top, tv)
                    elif nb >= 4:
                        h2 = nb // 2
                        nc.gpsimd.tensor_add(
                            xv[:, 0:h2:2, 0 : 2 * half],
                            xv[:, 0:h2:2, 0 : 2 * half],
                            tvb[:, 0:h2:2],
                        )
                        nc.vector.tensor_add(
                            xv[:, 1:h2:2, 0 : 2 * half],
                            xv[:, 1:h2:2, 0 : 2 * half],
                            tvb[:, 1:h2:2],
                        )
                        nc.vector.tensor_add(
                            xv[:, h2:nb, 0 : 2 * half],
                            xv[:, h2:nb, 0 : 2 * half],
                            tvb[:, h2:nb],
                        )
                    else:
                        # s == 11: nb == 2; gpsimd takes the first half of
                        # block 0's top (consumed by stage 12's late ops)
                        nc.gpsimd.tensor_add(
                            xv[:, 0:1, 0:half],
                            xv[:, 0:1, 0:half],
                            tvb[:, 0:1, 0:half],
                        )
                        nc.vector.tensor_add(
                            xv[:, 0:1, half : 2 * half],
                            xv[:, 0:1, half : 2 * half],
                            tvb[:, 0:1, half : 2 * half],
                        )
                        nc.vector.tensor_add(
                            xv[:, 1:2, 0 : 2 * half],
                            xv[:, 1:2, 0 : 2 * half],
                            tvb[:, 1:2],
                        )

                # ---- stage 12 in two column chunks; outputs DMA straight to
                # HBM as packed fp16 ----
                Q = N // 2
                for c in range(2):
                    top_c = x[:, c * Q : c * Q + Q]
                    bot_c = x[:, N + c * Q : N + c * Q + Q]
                    tw_c = pack[:, N + c * Q : N + c * Q + Q]
                    tv = t1[:, 0:Q]
                    emit_raw(nc, "CMUL_PACKED_ANT", tv, bot_c, tw_c)
                    nc.vector.tensor_sub(bot_c, top_c, tv)
                    hq = 2 + c
                    nc.sync.dma_start(
                        outT[rows, hq * Q : (hq + 1) * Q], x[:, hq * Q : (hq + 1) * Q]
                    )
                    nc.vector.tensor_add(top_c, top_c, tv)
                    nc.sync.dma_start(
                        outT[rows, c * Q : (c + 1) * Q], x[:, c * Q : (c + 1) * Q]
                    )

    nc.compile()
    patch_perf_bits(nc)
    return nc


# ===================== host glue =====================

_PERM = None


def _perm():
    global _PERM
    if _PERM is None:
        _PERM = np.arange(N) ^ (N // 2)
    return _PERM


def make_core_inputs(x: np.ndarray, weights: np.ndarray, core: int):
    sl = slice(core * DSH, (core + 1) * DSH)
    xp = x[_perm()][:, sl]  # pre-permuted rows
    xT = np.ascontiguousarray(xp.T).astype(np.float16)
    w = weights[: N // 2, sl].astype(np.float64)
    k = -(1.0 / N) * np.arange(N // 2, dtype=np.float64)
    rr = w * k[:, None]
    rr -= np.rint(rr)
    wT = np.ascontiguousarray(rr.T).astype(np.float32)
    wc = np.ascontiguousarray(wT[:, 1024:1025])
    return {"xT": xT, "wT": wT, "wc": wc}


def assemble_output(core_outs):
    full = np.empty((N, N), dtype=np.complex64)
    for c, r in enumerate(core_outs):
        oc = r["outT"].astype(np.float32).view(np.complex64)
        full[:, c * DSH : (c + 1) * DSH] = oc.T
    return full


_NC_CACHE = None


def get_nc():
    global _NC_CACHE
    if _NC_CACHE is None:
        _NC_CACHE = build_fft_nc()
    return _NC_CACHE


def make_in_maps(x: np.ndarray, weights: np.ndarray):
    x = np.asarray(x, dtype=np.float32)
    weights = np.asarray(weights, dtype=np.float32)
    in_maps = [make_core_inputs(x, weights, c) for c in range(NCORES)]
    return in_maps


def run_on_hw(x, weights, **spmd_kwargs):
    nc = get_nc()
    in_maps = make_in_maps(x, weights)
    res = run_bass_kernel_spmd(nc, in_maps, core_ids=list(range(NCORES)), **spmd_kwargs)
    return assemble_output(res.results), res


def kernel(x: np.ndarray, weights: np.ndarray) -> np.ndarray:
    out, _ = run_on_hw(x, weights)
    return out


# revision 13
# speedup vs baseline: 1.4095x; 1.0513x over previous
"""Trainium2 Bass kernel: data-dependent radix-2 FFT butterfly network.

out = FFT-like transform of x (4096x4096 f32 -> complex64); stage twiddles
are exp(-2j*pi*k/N * weights[k, :]) (learned, per-feature), N = 4096,
12 radix-2 stages, initial row permutation j ^ N/2.

Sharding: feature dim split across 8 NeuronCores (512 each) - the whole
network is elementwise along features, so no cross-core communication.

Per-core: features on partitions (4 groups of 128), FFT rows along the
free dim, x stored as packed (re, im) fp16 pairs. Each generic stage is
3 Vector-engine ops: a packed-complex-multiply custom DVE op (one
complex/cycle in 2X_1PORT mode) plus packed fp16 add/subs in 2x mode;
part of the butterfly add/sub work is offloaded to the otherwise-idle
GpSimd engine each stage. Stages 1-2 (real inputs, trivial twiddles) are
four fused quad ops that write the packed complex layout directly.
Twiddles are generated on-device by the Scalar engine's Sin LUT from
host-range-reduced phases. I/O is fp16 end-to-end: the host pre-permutes
(j ^ N/2), transposes and converts x to fp16, and the packed fp16 output
is converted to complex64 on the host. All tile pools are double-
buffered so consecutive 128-feature groups overlap.
"""

import math
import sys

import numpy as np

if "/opt/trn_rl_repo" not in sys.path:
    sys.path.insert(0, "/opt/trn_rl_repo")

import concourse.bacc as bacc
import concourse.bass as bass
import concourse.mybir as mybir
from concourse.bass_utils import run_bass_kernel_spmd
from concourse.tile import TileContext

F32 = mybir.dt.float32
F16 = mybir.dt.float16
AF = mybir.ActivationFunctionType
ALU = mybir.AluOpType

N = 4096
LOGN = 12
NCORES = 8
DSH = N // NCORES
NGROUPS = DSH // 128
PI = math.pi
TWO_PI = 2.0 * math.pi

# offload a slice of each stage's add pass to the GpSimd engine
# (measured net-negative: GpSimd's SBUF port contends with the DVE and slows
# every Vector op ~30%; keep off)
GP_OFFLOAD = False


# ===================== custom DVE ops =====================

import concourse.dve_ops as dve_ops
from concourse.dve_spec import Spec, Src0, Src1
from concourse.dve_uop import (
    AluInp,
    AluOp,
    DelayInp,
    DveOpSpec,
    InpSel,
    OutPath,
    OutSel,
    Trigger,
    UopConfig,
)

D = [
    AluInp.PREV_DELAY_0,
    AluInp.PREV_DELAY_1,
    AluInp.PREV_DELAY_2,
    AluInp.PREV_DELAY_3,
    AluInp.PREV_DELAY_4,
    AluInp.PREV_DELAY_5,
]


def _uop(inputs, req0, req1, trigger, next_uop, repeat=0):
    u = UopConfig()
    for lane, sel in enumerate(inputs, start=1):
        u.enable_input(sel, lane)
    u.require_inp0 = req0
    u.require_inp1 = req1
    u.trigger = trigger
    u.next_uop = next_uop
    u.repeat_count = repeat
    return u


_1STATE = dict(
    trigger=(Trigger.SRC_TENSOR_DONE, Trigger.NONE, Trigger.NONE),
    next_uop=(0, 0, 0),
)


# ---------------- CMUL (packed complex multiply, proven) ----------------


def _cmul_uop():
    u = _uop(
        [InpSel.SRC_0, InpSel.SRC_1, InpSel.SRC_0_HI, InpSel.SRC_1_HI],
        1,
        1,
        **_1STATE,
    )
    dp = u.datapath_config
    dp[0].enable_alu(AluOp.MULTIPLY, D[0], D[1])
    dp[0].pass_through_delay(0, 1, 2, 3)
    dp[1].enable_alu(AluOp.MULTIPLY, D[2], D[3])
    dp[1].pass_through_delay(0, 1, 2, 3)
    dp[1].enable_delay_from_src(DelayInp.PREV_ALU_OUT, 4)
    dp[2].enable_alu(AluOp.SUBTRACT, D[4], AluInp.PREV_ALU_OUT)
    dp[2].pass_through_delay(0, 1, 2, 3)
    dp[3].enable_alu(AluOp.MULTIPLY, D[0], D[3])
    dp[3].pass_through_delay(1, 2)
    dp[3].enable_delay_from_src(DelayInp.PREV_ALU_OUT, 4)
    dp[4].enable_alu(AluOp.MULTIPLY, D[2], D[1])
    dp[4].pass_through_delay(4)
    dp[4].enable_delay_from_src(DelayInp.PREV_ALU_OUT, 0)
    dp[5].enable_alu(AluOp.ADD, D[0], AluInp.PREV_ALU_OUT)
    dp[5].pass_through_delay(4)
    dp[6].pass_through_alu()
    dp[6].pass_through_delay(4)
    dp[7].pass_through_alu()
    dp[7].pass_through_delay(4)
    u.enable_output(OutSel.DELAY_4, OutPath.WR0_LO)
    u.enable_output(OutSel.ALU_OUT, OutPath.WR0_HI)
    return u


def _cmul_reference(in0, in1, c0, c1, c2):
    a = in0.astype(np.float32)
    b = np.broadcast_to(in1, in0.shape).astype(np.float32)
    out = np.empty_like(a)
    ar, ai = a[..., 0::2], a[..., 1::2]
    br, bi = b[..., 0::2], b[..., 1::2]
    out[..., 0::2] = ar * br - ai * bi
    out[..., 1::2] = ar * bi + ai * br
    return out


# ---------------- stage-1+2 fused quad ops ----------------
# Each quad of 4 consecutive (pre-permuted) rows (a, b, c, d) produces the
# complex stage-2 outputs written straight into the packed (re, im) layout:
#   y0 = (a+b)+(c+d)           im 0        -> word 4q+0
#   y1 = ((a-b)+C0*(c-d),  C1*(c-d))       -> word 4q+1
#   y2 = (a+b)-(c+d)           im 0        -> word 4q+2
#   y3 = ((a-b)-C0*(c-d), -C1*(c-d))       -> word 4q+3  (C1 passed negated)
# src0 = (a,b) even words of the real plane, src1 = (c,d) odd words.


def _q02_uop(sub: bool):
    u = _uop(
        [InpSel.SRC_0, InpSel.SRC_0_HI, InpSel.SRC_1, InpSel.SRC_1_HI, InpSel.ZERO],
        1,
        1,
        **_1STATE,
    )
    dp = u.datapath_config
    dp[0].enable_alu(AluOp.ADD, D[0], D[1])  # t0 = a+b
    dp[0].pass_through_delay(2, 3, 4)
    dp[1].enable_alu(AluOp.ADD, D[2], D[3])  # t1 = c+d
    dp[1].enable_delay_from_src(DelayInp.PREV_ALU_OUT, 0)  # t0
    dp[1].pass_through_delay(4)
    if sub:
        dp[2].enable_alu(AluOp.SUBTRACT, D[0], AluInp.PREV_ALU_OUT)  # t0-t1
    else:
        dp[2].enable_alu(AluOp.ADD, AluInp.PREV_ALU_OUT, D[0])  # t0+t1
    dp[2].pass_through_delay(4)
    for k in (3, 4, 5, 6, 7):
        dp[k].pass_through_alu()
        dp[k].pass_through_delay(4)
    u.enable_output(OutSel.ALU_OUT, OutPath.WR0_LO)
    u.enable_output(OutSel.DELAY_4, OutPath.WR0_HI)  # zero im
    return u


def _q13_uop(sub: bool):
    u = _uop(
        [
            InpSel.SRC_0,
            InpSel.SRC_0_HI,
            InpSel.SRC_1,
            InpSel.SRC_1_HI,
            InpSel.CONST_0,
            InpSel.ZERO,
        ],
        1,
        1,
        **_1STATE,
    )
    dp = u.datapath_config
    # chains: 0=a 1=b 2=c->p 3=d 4=C0 5=zero
    dp[0].enable_alu(AluOp.SUBTRACT, D[2], D[3])  # u = c-d
    dp[0].pass_through_delay(0, 1, 4, 5)
    dp[1].enable_alu(AluOp.MULTIPLY, AluInp.PREV_ALU_OUT, D[4])  # p = C0*u
    dp[1].pass_through_delay(0, 1, 5)
    dp[2].enable_alu(AluOp.SUBTRACT, D[0], D[1])  # t2 = a-b
    dp[2].enable_delay_from_src(DelayInp.PREV_ALU_OUT, 2)  # p
    dp[2].pass_through_delay(5)
    if sub:
        dp[3].enable_alu(AluOp.SUBTRACT, AluInp.PREV_ALU_OUT, D[2])  # t2 - p
    else:
        dp[3].enable_alu(AluOp.ADD, AluInp.PREV_ALU_OUT, D[2])  # t2 + p
    dp[3].pass_through_delay(5)
    for k in (4, 5, 6, 7):
        dp[k].pass_through_alu()
        dp[k].pass_through_delay(5)
    u.enable_output(OutSel.ALU_OUT, OutPath.WR0_LO)  # y re
    u.enable_output(OutSel.DELAY_5, OutPath.WR0_HI)  # zero im
    return u


# compact im pairs (q, -q), q = C0*(c-d); ACT scatters into the im slots
def _qim_uop():
    u = _uop(
        [
            InpSel.SRC_0,
            InpSel.SRC_0_HI,
            InpSel.SRC_1,
            InpSel.SRC_1_HI,
            InpSel.CONST_0,
            InpSel.ZERO,
        ],
        1,
        1,
        **_1STATE,
    )
    dp = u.datapath_config
    dp[0].enable_alu(AluOp.SUBTRACT, D[2], D[3])  # u = c-d
    dp[0].pass_through_delay(4, 5)
    dp[1].enable_alu(AluOp.MULTIPLY, AluInp.PREV_ALU_OUT, D[4])  # q
    dp[1].pass_through_delay(5)
    dp[2].enable_alu(AluOp.SUBTRACT, D[5], AluInp.PREV_ALU_OUT)  # -q
    dp[2].enable_delay_from_src(DelayInp.PREV_ALU_OUT, 0)  # q
    for k in (3, 4, 5, 6, 7):
        dp[k].pass_through_alu()
        dp[k].pass_through_delay(0)
    u.enable_output(OutSel.DELAY_0, OutPath.WR0_LO)  # q
    u.enable_output(OutSel.ALU_OUT, OutPath.WR0_HI)  # -q
    return u


def _qim_reference(in0, in1, c0, c1, c2):
    b = np.asarray(in1).astype(np.float32)
    ss = np.asarray(c0, np.float32).reshape(-1, *([1] * (b.ndim - 1)))
    q = ss * (b[..., 0::2] - b[..., 1::2])
    out = np.empty_like(b)
    out[..., 0::2] = q
    out[..., 1::2] = -q
    return out


def _q0_reference(in0, in1, c0, c1, c2):
    a = in0.astype(np.float32)
    b = np.asarray(in1).astype(np.float32)
    out = np.empty_like(a)
    out[..., 0::2] = (a[..., 0::2] + a[..., 1::2]) + (b[..., 0::2] + b[..., 1::2])
    out[..., 1::2] = 0.0
    return out


def _q2_reference(in0, in1, c0, c1, c2):
    a = in0.astype(np.float32)
    b = np.asarray(in1).astype(np.float32)
    out = np.empty_like(a)
    out[..., 0::2] = (a[..., 0::2] + a[..., 1::2]) - (b[..., 0::2] + b[..., 1::2])
    out[..., 1::2] = 0.0
    return out


def _q13_reference(sub):
    def ref(in0, in1, c0, c1, c2):
        a = in0.astype(np.float32)
        b = np.asarray(in1).astype(np.float32)
        cc = np.asarray(c0, np.float32).reshape(-1, *([1] * (a.ndim - 1)))
        t2 = a[..., 0::2] - a[..., 1::2]
        u = b[..., 0::2] - b[..., 1::2]
        out = np.empty_like(a)
        out[..., 0::2] = t2 - cc * u if sub else t2 + cc * u
        out[..., 1::2] = 0.0
        return out

    return ref


# ---------------- registry ----------------


class RawDveOp:
    def __init__(self, name, mk_all, rd1_en, perf_max, reference):
        self.name = name
        self.subdim = False
        self.spec = Spec(body=Src0 * Src1 if rd1_en else Src0, reference=reference)
        self.rd1_en = rd1_en
        self.perf_max = perf_max
        self._mk = mk_all
        self._cache = {}

    def compile(self, ver):
        if ver in self._cache:
            return self._cache[ver]
        kw = self._mk()
        spec = DveOpSpec(
            name=self.name,
            opcode=dve_ops.get_dve_sub_opcode(self.name),
            perf_max=self.perf_max,
            rd1_en=self.rd1_en,
            **kw,
        )
        spec.validate(ver)
        self._cache[ver] = spec
        return spec


RAW_OPS = {}


def register_raw_ops():
    if RAW_OPS:
        return RAW_OPS
    defs = [
        RawDveOp(
            "CMUL_PACKED_ANT",
            lambda: dict(uops=[_cmul_uop()], uops_2x=[_cmul_uop()]),
            True,
            1,
            _cmul_reference,
        ),
        RawDveOp(
            "QUAD0_ANT",
            lambda: dict(uops=[_q02_uop(False)], uops_2x=[_q02_uop(False)]),
            True,
            1,
            _q0_reference,
        ),
        RawDveOp(
            "QUAD2_ANT",
            lambda: dict(uops=[_q02_uop(True)], uops_2x=[_q02_uop(True)]),
            True,
            1,
            _q2_reference,
        ),
        RawDveOp(
            "QUAD1_ANT",
            lambda: dict(uops=[_q13_uop(False)], uops_2x=[_q13_uop(False)]),
            True,
            1,
            _q13_reference(False),
        ),
        RawDveOp(
            "QUAD3_ANT",
            lambda: dict(uops=[_q13_uop(True)], uops_2x=[_q13_uop(True)]),
            True,
            1,
            _q13_reference(True),
        ),
        RawDveOp(
            "QIM_ANT",
            lambda: dict(uops=[_qim_uop()], uops_2x=[_qim_uop()]),
            True,
            1,
            _qim_reference,
        ),
    ]
    for op in defs:
        if op.name not in dve_ops._SUB_OPCODE_FOR_NAME:
            dve_ops.OPS.append(op)
            row = dve_ops._CUSTOM_DVE_ROW_BASE + len(dve_ops.OPS) - 1
            assert row < 0x20
            dve_ops._SUB_OPCODE_FOR_NAME[op.name] = row
            dve_ops.CUSTOM_DVE_SPECS[op.name] = op.spec
        RAW_OPS[op.name] = op
    return RAW_OPS


def emit_raw(nc, name, out, in0, in1=None, s0=None, s1=None):
    import concourse.bass_isa as bass_isa

    ops = register_raw_ops()
    op = ops[name]
    v = nc.vector
    if op.name not in nc.m.ant_custom_dve_ops:
        nc.m.ant_custom_dve_ops = sorted({*nc.m.ant_custom_dve_ops, op.name})
    shape = (
        bass_isa.CustomDveShape.STT
        if in1 is not None
        else bass_isa.CustomDveShape.TTSS
    )
    isa_opcode = nc.isa.Opcode[
        f"NEURON_ISA_TPB_OPCODE_CUSTOM_DVE_ANT_{shape.slot()}"
    ].value
    imm = mybir.ImmediateValue(dtype=mybir.dt.float32, value=0.0)
    s0a = v.lower_ap(s0, for_isa=True) if s0 is not None else imm
    s1a = v.lower_ap(s1, for_isa=True) if s1 is not None else imm
    ins = [v.lower_ap(in0, for_isa=True)]
    if in1 is not None:
        ins.append(v.lower_ap(in1, for_isa=True))
    ins += [s0a, s1a]
    return v.add_instruction(
        bass_isa.InstCustomDveAnt(
            name=nc.get_next_instruction_name(),
            op_name=op.name,
            rd1_en=op.rd1_en,
            subdim=0,
            imm2=0.0,
            shape=shape,
            row=dve_ops.get_dve_sub_opcode(op.name),
            isa_opcode=isa_opcode,
            ins=ins,
            outs=[v.lower_ap(out, for_isa=True)],
        )
    )


def patch_perf_bits(nc):
    ops = register_raw_ops()
    n = 0
    for fn in nc.m.functions:
        for blk in fn.blocks:
            for inst in blk.instructions:
                nm = getattr(inst, "op_name", None)
                if nm in ops:
                    bb = bytearray(bytes(inst.instr))
                    bb[36] |= ops[nm].perf_max << 6
                    inst.instr = bytes(bb)
                    n += 1
    return n


# ===================== kernel builder =====================


def build_fft_nc():
    register_raw_ops()
    nc = bacc.Bacc()

    xT = nc.dram_tensor("xT", [DSH, N], F16, kind="ExternalInput")
    # wT rows: [0:2048) = reduced phase r, [2048:4096) = |r| (host-prepared)
    wT = nc.dram_tensor("wT", [DSH, N], F32, kind="ExternalInput")
    wc = nc.dram_tensor("wc", [DSH, 2], F32, kind="ExternalInput")  # (rc, |rc|)
    outT = nc.dram_tensor("outT", [DSH, 2 * N], F16, kind="ExternalOutput")

    # const AP: pi/2 bias for the cos path
    HPI = float(np.float32(PI / 2))
    chp = nc.alloc_sbuf_tensor("const-f32-hpi", [128, 1], F32)
    nc.gpsimd.memset(chp.ap(), HPI)
    nc.const_aps.aps[(F32, HPI)] = chp.ap()
    nc.all_engine_barrier()

    with TileContext(nc) as tc:
        with (
            tc.tile_pool(name="xr", bufs=2) as xrpool,
            tc.tile_pool(name="xbuf", bufs=2) as xpool,
            tc.tile_pool(name="tmp", bufs=2) as tpool,
            tc.tile_pool(name="tw", bufs=2) as twpool,
            tc.tile_pool(name="ph", bufs=2) as ppool,
            tc.tile_pool(name="col", bufs=2) as colpool,
        ):
            for g in range(NGROUPS):
                rows = slice(g * 128, (g + 1) * 128)

                # ---- tiny stage-2 column phases first (unblocks quads) ----
                rc = colpool.tile([128, 2], F32, tag="rc")
                nc.sync.dma_start(rc[:], wc[rows, :])
                cols = colpool.tile([128, 2], F32, tag="cols")
                nc.scalar.activation(
                    cols[:, 0:1], rc[:, 1:2], AF.Sin, scale=-TWO_PI, bias=HPI
                )  # c = cos
                nc.scalar.activation(cols[:, 1:2], rc[:, 0:1], AF.Sin, scale=TWO_PI)

                # ---- phases arrive host-reduced: wT[p, 0:2048] = r in
                # [-0.5, 0.5] with sin(2pi*r) = sin(phi); wT[p, 2048:] = |r| ----
                r = ppool.tile([128, N], F32, tag="r")
                nc.sync.dma_start(r[:], wT[rows, :])

                # ---- interleaved twiddle packs: stage s at [2*half, 4*half).
                # All scalar work here is vector-independent: keep it ahead of
                # the im-scatter in the in-order scalar queue. ----
                pack = twpool.tile([128, 2 * N], F16, tag="pack")
                for s in range(3, LOGN + 1):
                    half = 1 << (s - 1)
                    stride = N >> s
                    src_im = r[:, 0 : N // 2 : stride]
                    src_re = r[:, N // 2 : N : stride]
                    nc.scalar.activation(
                        pack[:, 2 * half : 4 * half : 2],
                        src_re,
                        AF.Sin,
                        scale=-TWO_PI,
                        bias=HPI,
                    )
                    nc.scalar.activation(
                        pack[:, 2 * half + 1 : 4 * half : 2],
                        src_im,
                        AF.Sin,
                        scale=TWO_PI,
                    )

                # ---- x real plane (host pre-permuted fp16) ----
                xr = xrpool.tile([128, N], F16, tag="xplane")
                nc.sync.dma_start(xr[:], xT[rows, :])

                # ---- stages 1+2: fused quad ops -> packed complex x.
                # QIM first so its ACT im-scatter overlaps Q0/Q2. ----
                x = xpool.tile([128, 2 * N], F16, tag="x")
                xr4 = xr[:].rearrange("p (b f) -> p b f", f=4)
                src0 = xr4[:, :, 0:2]
                src1 = xr4[:, :, 2:4]
                x8 = x[:].rearrange("p (b f) -> p b f", f=8)
                imc = tpool.tile([128, N // 2], F16, tag="imc")
                imc2 = imc[:].rearrange("p (b f) -> p b f", f=2)
                emit_raw(nc, "QIM_ANT", imc2, src0, src1, s0=cols[:, 1:2])
                emit_raw(
                    nc, "QUAD1_ANT", x8[:, :, 2:4], src0, src1, s0=cols[:, 0:1]
                )
                emit_raw(
                    nc, "QUAD3_ANT", x8[:, :, 6:8], src0, src1, s0=cols[:, 0:1]
                )
                nc.scalar.activation(x8[:, :, 3:8:4], imc2, AF.Copy)
                emit_raw(nc, "QUAD0_ANT", x8[:, :, 0:2], src0, src1)
                emit_raw(nc, "QUAD2_ANT", x8[:, :, 4:6], src0, src1)

                t1 = tpool.tile([128, N], F16, tag="t1")  # packed cmul temp

                # ---- stages 3..11: packed generic. A 512-word slice of the
                # add pass runs on GpSimd: even-index blocks in the first
                # half -- next stage's CMUL (odd blocks) never reads them,
                # only the later sub/add do, so GpSimd overlaps fully. ----
                for s in range(3, LOGN):
                    step = 1 << s
                    half = step // 2
                    nb = N // step

                    xv = x[:].rearrange("p (b stc) -> p b stc", stc=2 * step)
                    top = xv[:, :, 0 : 2 * half]
                    bot = xv[:, :, 2 * half : 2 * step]
                    tw = (
                        pack[:, 2 * half : 4 * half]
                        .unsqueeze(1)
                        .broadcast_to([128, nb, 2 * half])
                    )
                    tv = t1[:, 0 : nb * 2 * half]
                    tvb = tv.rearrange("p (b h) -> p b h", h=2 * half)
                    if nb > 1:
                        tv = tvb
                    emit_raw(nc, "CMUL_PACKED_ANT", tv, bot, tw)
                    nc.vector.tensor_sub(bot, top, tv)
                    if not GP_OFFLOAD:
                        nc.vector.tensor_add(top, top, tv)
                    elif nb >= 4:
                        h2 = nb // 2
                        nc.gpsimd.tensor_add(
                            xv[:, 0:h2:2, 0 : 2 * half],
                            xv[:, 0:h2:2, 0 : 2 * half],
                            tvb[:, 0:h2:2],
                        )
                        nc.vector.tensor_add(
                            xv[:, 1:h2:2, 0 : 2 * half],
                            xv[:, 1:h2:2, 0 : 2 * half],
                            tvb[:, 1:h2:2],
                        )
                        nc.vector.tensor_add(
                            xv[:, h2:nb, 0 : 2 * half],
                            xv[:, h2:nb, 0 : 2 * half],
                            tvb[:, h2:nb],
                        )
                    else:
                        # s == 11: nb == 2; gpsimd takes the first half of
                        # block 0's top (consumed by stage 12's late ops)
                        nc.gpsimd.tensor_add(
                            xv[:, 0:1, 0:half],
                            xv[:, 0:1, 0:half],
                            tvb[:, 0:1, 0:half],
                        )
                        nc.vector.tensor_add(
                            xv[:, 0:1, half : 2 * half],
                            xv[:, 0:1, half : 2 * half],
                            tvb[:, 0:1, half : 2 * half],
                        )
                        nc.vector.tensor_add(
                            xv[:, 1:2, 0 : 2 * half],
                            xv[:, 1:2, 0 : 2 * half],
                            tvb[:, 1:2],
                        )

                # ---- stage 12 in two column chunks; outputs DMA straight to
                # HBM as packed fp16 ----
                Q = N // 2
                for c in range(2):
                    top_c = x[:, c * Q : c * Q + Q]
                    bot_c = x[:, N + c * Q : N + c * Q + Q]
                    tw_c = pack[:, N + c * Q : N + c * Q + Q]
                    tv = t1[:, 0:Q]
                    emit_raw(nc, "CMUL_PACKED_ANT", tv, bot_c, tw_c)
                    nc.vector.tensor_sub(bot_c, top_c, tv)
                    hq = 2 + c
                    nc.sync.dma_start(
                        outT[rows, hq * Q : (hq + 1) * Q], x[:, hq * Q : (hq + 1) * Q]
                    )
                    nc.vector.tensor_add(top_c, top_c, tv)
                    nc.sync.dma_start(
                        outT[rows, c * Q : (c + 1) * Q], x[:, c * Q : (c + 1) * Q]
                    )

    nc.compile()
    patch_perf_bits(nc)
    return nc


# ===================== host glue =====================

_PERM = None


def _perm():
    global _PERM
    if _PERM is None:
        _PERM = np.arange(N) ^ (N // 2)
    return _PERM


def make_core_inputs(x: np.ndarray, weights: np.ndarray, core: int):
    sl = slice(core * DSH, (core + 1) * DSH)
    xp = x[_perm()][:, sl]  # pre-permuted rows
    xT = np.ascontiguousarray(xp.T).astype(np.float16)
    w = weights[: N // 2, sl].astype(np.float64)
    k = -(1.0 / N) * np.arange(N // 2, dtype=np.float64)
    rr = w * k[:, None]
    rr -= np.rint(rr)
    rT = np.ascontiguousarray(rr.T).astype(np.float32)
    wT = np.concatenate([rT, np.abs(rT)], axis=1)
    wc = np.ascontiguousarray(wT[:, [1024, 2048 + 1024]])
    return {"xT": xT, "wT": wT, "wc": wc}


def assemble_output(core_outs):
    full = np.empty((N, N), dtype=np.complex64)
    for c, r in enumerate(core_outs):
        oc = r["outT"].astype(np.float32).view(np.complex64)
        full[:, c * DSH : (c + 1) * DSH] = oc.T
    return full


_NC_CACHE = None


def get_nc():
    global _NC_CACHE
    if _NC_CACHE is None:
        _NC_CACHE = build_fft_nc()
    return _NC_CACHE


def make_in_maps(x: np.ndarray, weights: np.ndarray):
    x = np.asarray(x, dtype=np.float32)
    weights = np.asarray(weights, dtype=np.float32)
    in_maps = [make_core_inputs(x, weights, c) for c in range(NCORES)]
    return in_maps


def run_on_hw(x, weights, **spmd_kwargs):
    nc = get_nc()
    in_maps = make_in_maps(x, weights)
    res = run_bass_kernel_spmd(nc, in_maps, core_ids=list(range(NCORES)), **spmd_kwargs)
    return assemble_output(res.results), res


def kernel(x: np.ndarray, weights: np.ndarray) -> np.ndarray:
    out, _ = run_on_hw(x, weights)
    return out


# revision 19
# speedup vs baseline: 1.4120x; 1.0017x over previous
"""Trainium2 Bass kernel: data-dependent radix-2 FFT butterfly network.

out = FFT-like transform of x (4096x4096 f32 -> complex64); stage twiddles
are exp(-2j*pi*k/N * weights[k, :]) (learned, per-feature), N = 4096,
12 radix-2 stages, initial row permutation j ^ N/2.

Sharding: feature dim split across 8 NeuronCores (512 each) - the whole
network is elementwise along features, so no cross-core communication.

Per-core: features on partitions (4 groups of 128), FFT rows along the
free dim, x stored as packed (re, im) fp16 pairs. Each generic stage is
3 Vector-engine ops: a packed-complex-multiply custom DVE op (one
complex/cycle in 2X_1PORT mode) plus packed fp16 add/subs in 2x mode;
part of the butterfly add/sub work is offloaded to the otherwise-idle
GpSimd engine each stage. Stages 1-2 (real inputs, trivial twiddles) are
four fused quad ops that write the packed complex layout directly.
Twiddles are generated on-device by the Scalar engine's Sin LUT from
host-range-reduced phases. I/O is fp16 end-to-end: the host pre-permutes
(j ^ N/2), transposes and converts x to fp16, and the packed fp16 output
is converted to complex64 on the host. All tile pools are double-
buffered so consecutive 128-feature groups overlap.
"""

import math
import sys

import numpy as np

if "/opt/trn_rl_repo" not in sys.path:
    sys.path.insert(0, "/opt/trn_rl_repo")

import concourse.bacc as bacc
import concourse.bass as bass
import concourse.mybir as mybir
from concourse.bass_utils import run_bass_kernel_spmd
from concourse.tile import TileContext

F32 = mybir.dt.float32
F16 = mybir.dt.float16
AF = mybir.ActivationFunctionType
ALU = mybir.AluOpType

N = 4096
LOGN = 12
NCORES = 8
DSH = N // NCORES
NGROUPS = DSH // 128
PI = math.pi
TWO_PI = 2.0 * math.pi

# offload a slice of each stage's add pass to the GpSimd engine
# (measured net-negative: GpSimd's SBUF port contends with the DVE and slows
# every Vector op ~30%; keep off)
GP_OFFLOAD = False
# 2-state fused quad ops (QA/QB) with 8B-contiguous writes vs four 1-state
# ops whose 16B-strided writes run at half throughput (multi-state custom
# machines fault the DVE; keep off)
USE_QAB = False


# ===================== custom DVE ops =====================

import concourse.dve_ops as dve_ops
from concourse.dve_spec import Spec, Src0, Src1
from concourse.dve_uop import (
    AluInp,
    AluOp,
    DelayInp,
    DveOpSpec,
    InpSel,
    OutPath,
    OutSel,
    Trigger,
    UopConfig,
)

D = [
    AluInp.PREV_DELAY_0,
    AluInp.PREV_DELAY_1,
    AluInp.PREV_DELAY_2,
    AluInp.PREV_DELAY_3,
    AluInp.PREV_DELAY_4,
    AluInp.PREV_DELAY_5,
]


def _uop(inputs, req0, req1, trigger, next_uop, repeat=0):
    u = UopConfig()
    for lane, sel in enumerate(inputs, start=1):
        u.enable_input(sel, lane)
    u.require_inp0 = req0
    u.require_inp1 = req1
    u.trigger = trigger
    u.next_uop = next_uop
    u.repeat_count = repeat
    return u


_1STATE = dict(
    trigger=(Trigger.SRC_TENSOR_DONE, Trigger.NONE, Trigger.NONE),
    next_uop=(0, 0, 0),
)


# ---------------- CMUL (packed complex multiply, proven) ----------------


def _cmul_uop():
    u = _uop(
        [InpSel.SRC_0, InpSel.SRC_1, InpSel.SRC_0_HI, InpSel.SRC_1_HI],
        1,
        1,
        **_1STATE,
    )
    dp = u.datapath_config
    dp[0].enable_alu(AluOp.MULTIPLY, D[0], D[1])
    dp[0].pass_through_delay(0, 1, 2, 3)
    dp[1].enable_alu(AluOp.MULTIPLY, D[2], D[3])
    dp[1].pass_through_delay(0, 1, 2, 3)
    dp[1].enable_delay_from_src(DelayInp.PREV_ALU_OUT, 4)
    dp[2].enable_alu(AluOp.SUBTRACT, D[4], AluInp.PREV_ALU_OUT)
    dp[2].pass_through_delay(0, 1, 2, 3)
    dp[3].enable_alu(AluOp.MULTIPLY, D[0], D[3])
    dp[3].pass_through_delay(1, 2)
    dp[3].enable_delay_from_src(DelayInp.PREV_ALU_OUT, 4)
    dp[4].enable_alu(AluOp.MULTIPLY, D[2], D[1])
    dp[4].pass_through_delay(4)
    dp[4].enable_delay_from_src(DelayInp.PREV_ALU_OUT, 0)
    dp[5].enable_alu(AluOp.ADD, D[0], AluInp.PREV_ALU_OUT)
    dp[5].pass_through_delay(4)
    dp[6].pass_through_alu()
    dp[6].pass_through_delay(4)
    dp[7].pass_through_alu()
    dp[7].pass_through_delay(4)
    u.enable_output(OutSel.DELAY_4, OutPath.WR0_LO)
    u.enable_output(OutSel.ALU_OUT, OutPath.WR0_HI)
    return u


def _cmul_reference(in0, in1, c0, c1, c2):
    a = in0.astype(np.float32)
    b = np.broadcast_to(in1, in0.shape).astype(np.float32)
    out = np.empty_like(a)
    ar, ai = a[..., 0::2], a[..., 1::2]
    br, bi = b[..., 0::2], b[..., 1::2]
    out[..., 0::2] = ar * br - ai * bi
    out[..., 1::2] = ar * bi + ai * br
    return out


# ---------------- stage-1+2 fused quad ops ----------------
# Each quad of 4 consecutive (pre-permuted) rows (a, b, c, d) produces the
# complex stage-2 outputs written straight into the packed (re, im) layout:
#   y0 = (a+b)+(c+d)           im 0        -> word 4q+0
#   y1 = ((a-b)+C0*(c-d),  C1*(c-d))       -> word 4q+1
#   y2 = (a+b)-(c+d)           im 0        -> word 4q+2
#   y3 = ((a-b)-C0*(c-d), -C1*(c-d))       -> word 4q+3  (C1 passed negated)
# src0 = (a,b) even words of the real plane, src1 = (c,d) odd words.


def _q02_uop(sub: bool):
    u = _uop(
        [InpSel.SRC_0, InpSel.SRC_0_HI, InpSel.SRC_1, InpSel.SRC_1_HI, InpSel.ZERO],
        1,
        1,
        **_1STATE,
    )
    dp = u.datapath_config
    dp[0].enable_alu(AluOp.ADD, D[0], D[1])  # t0 = a+b
    dp[0].pass_through_delay(2, 3, 4)
    dp[1].enable_alu(AluOp.ADD, D[2], D[3])  # t1 = c+d
    dp[1].enable_delay_from_src(DelayInp.PREV_ALU_OUT, 0)  # t0
    dp[1].pass_through_delay(4)
    if sub:
        dp[2].enable_alu(AluOp.SUBTRACT, D[0], AluInp.PREV_ALU_OUT)  # t0-t1
    else:
        dp[2].enable_alu(AluOp.ADD, AluInp.PREV_ALU_OUT, D[0])  # t0+t1
    dp[2].pass_through_delay(4)
    for k in (3, 4, 5, 6, 7):
        dp[k].pass_through_alu()
        dp[k].pass_through_delay(4)
    u.enable_output(OutSel.ALU_OUT, OutPath.WR0_LO)
    u.enable_output(OutSel.DELAY_4, OutPath.WR0_HI)  # zero im
    return u


def _q13_uop(sub: bool):
    u = _uop(
        [
            InpSel.SRC_0,
            InpSel.SRC_0_HI,
            InpSel.SRC_1,
            InpSel.SRC_1_HI,
            InpSel.CONST_0,
            InpSel.ZERO,
        ],
        1,
        1,
        **_1STATE,
    )
    dp = u.datapath_config
    # chains: 0=a 1=b 2=c->p 3=d 4=C0 5=zero
    dp[0].enable_alu(AluOp.SUBTRACT, D[2], D[3])  # u = c-d
    dp[0].pass_through_delay(0, 1, 4, 5)
    dp[1].enable_alu(AluOp.MULTIPLY, AluInp.PREV_ALU_OUT, D[4])  # p = C0*u
    dp[1].pass_through_delay(0, 1, 5)
    dp[2].enable_alu(AluOp.SUBTRACT, D[0], D[1])  # t2 = a-b
    dp[2].enable_delay_from_src(DelayInp.PREV_ALU_OUT, 2)  # p
    dp[2].pass_through_delay(5)
    if sub:
        dp[3].enable_alu(AluOp.SUBTRACT, AluInp.PREV_ALU_OUT, D[2])  # t2 - p
    else:
        dp[3].enable_alu(AluOp.ADD, AluInp.PREV_ALU_OUT, D[2])  # t2 + p
    dp[3].pass_through_delay(5)
    for k in (4, 5, 6, 7):
        dp[k].pass_through_alu()
        dp[k].pass_through_delay(5)
    u.enable_output(OutSel.ALU_OUT, OutPath.WR0_LO)  # y re
    u.enable_output(OutSel.DELAY_5, OutPath.WR0_HI)  # zero im
    return u


# ---- 2-state fused quad ops: QA emits words (y0re,0),(y1re,0) on
# consecutive cycles (one 8B-contiguous dst run per quad); QB same for
# y2/y3. State A consumes the quad, computes everything, emits word 0 and
# parks y1re in block 6's out-flop; state B emits word 1 from CURR. ----


def _qab_A(sub: bool, next_b: int):
    u = _uop(
        [
            InpSel.SRC_0,
            InpSel.SRC_0_HI,
            InpSel.SRC_1,
            InpSel.SRC_1_HI,
            InpSel.CONST_0,
        ],
        1,
        1,
        (Trigger.COUNT, Trigger.NONE, Trigger.NONE),
        (next_b, 0, 0),
        repeat=1,
    )
    op1 = AluOp.SUBTRACT if sub else AluOp.ADD
    dp = u.datapath_config
    # chains: 0=a 1=b 2=c->t1->y02 3=d->p 4=C0 5<-t0
    dp[0].enable_alu(AluOp.ADD, D[0], D[1])  # t0 = a+b
    dp[0].pass_through_delay(0, 1, 2, 3, 4)
    dp[1].enable_alu(AluOp.ADD, D[2], D[3])  # t1 = c+d
    dp[1].enable_delay_from_src(DelayInp.PREV_ALU_OUT, 5)  # t0
    dp[1].pass_through_delay(0, 1, 2, 3, 4)
    dp[2].enable_alu(AluOp.SUBTRACT, D[2], D[3])  # u = c-d
    dp[2].enable_delay_from_src(DelayInp.PREV_ALU_OUT, 2)  # t1
    dp[2].pass_through_delay(0, 1, 4, 5)
    dp[3].enable_alu(AluOp.MULTIPLY, AluInp.PREV_ALU_OUT, D[4])  # p = C0*u
    dp[3].pass_through_delay(0, 1, 2, 5)
    dp[4].enable_alu(op1, D[5], D[2])  # y02 = t0 (+/-) t1
    dp[4].enable_delay_from_src(DelayInp.PREV_ALU_OUT, 3)  # p
    dp[4].pass_through_delay(0, 1)
    dp[5].enable_alu(AluOp.SUBTRACT, D[0], D[1])  # t2 = a-b
    dp[5].enable_delay_from_src(DelayInp.PREV_ALU_OUT, 2)  # y02
    dp[5].pass_through_delay(3)
    dp[6].enable_alu(op1, AluInp.PREV_ALU_OUT, D[3])  # y13 = t2 (+/-) p [CURR]
    dp[6].pass_through_delay(2)
    dp[7].enable_alu(AluOp.SUBTRACT, AluInp.PREV_ALU_OUT, AluInp.PREV_ALU_OUT)
    dp[7].pass_through_delay(2)
    u.enable_output(OutSel.DELAY_2, OutPath.WR0_LO)  # y02
    u.enable_output(OutSel.ALU_OUT, OutPath.WR0_HI)  # 0
    return u


def _qab_B(next_a: int):
    u = _uop(
        [InpSel.SRC_0, InpSel.SRC_0_HI, InpSel.SRC_1, InpSel.SRC_1_HI],
        0,
        0,
        (Trigger.DST_TENSOR_DONE, Trigger.COUNT, Trigger.NONE),
        (0, next_a, 0),
        repeat=1,
    )
    dp = u.datapath_config
    dp[6].enable_alu(AluOp.BYPASS, AluInp.CURR_ALU_OUT)  # y13
    dp[7].enable_alu(AluOp.SUBTRACT, AluInp.PREV_ALU_OUT, AluInp.PREV_ALU_OUT)
    dp[7].enable_delay_from_src(DelayInp.PREV_ALU_OUT, 0)  # y13
    u.enable_output(OutSel.DELAY_0, OutPath.WR0_LO)  # y13
    u.enable_output(OutSel.ALU_OUT, OutPath.WR0_HI)  # 0
    return u


def _qab_uops(sub: bool):
    return [_qab_A(sub, 1), _qab_B(2), _qab_A(sub, 1)]


def _qab_reference(sub):
    def ref(in0, in1, c0, c1, c2):
        a = in0.astype(np.float32)
        b = np.asarray(in1).astype(np.float32)
        cc = np.asarray(c0, np.float32).reshape(-1, *([1] * (a.ndim - 1)))
        t0 = a[..., 0::2] + a[..., 1::2]
        t2 = a[..., 0::2] - a[..., 1::2]
        t1 = b[..., 0::2] + b[..., 1::2]
        u = b[..., 0::2] - b[..., 1::2]
        p = cc * u
        out = np.empty(a.shape[:-1] + (2 * a.shape[-1],), np.float32)
        out[..., 0::4] = t0 - t1 if sub else t0 + t1
        out[..., 1::4] = 0.0
        out[..., 2::4] = t2 - p if sub else t2 + p
        out[..., 3::4] = 0.0
        return out

    return ref


# compact im pairs (q, -q), q = C0*(c-d); ACT scatters into the im slots
def _qim_uop():
    u = _uop(
        [
            InpSel.SRC_0,
            InpSel.SRC_0_HI,
            InpSel.SRC_1,
            InpSel.SRC_1_HI,
            InpSel.CONST_0,
            InpSel.ZERO,
        ],
        1,
        1,
        **_1STATE,
    )
    dp = u.datapath_config
    dp[0].enable_alu(AluOp.SUBTRACT, D[2], D[3])  # u = c-d
    dp[0].pass_through_delay(4, 5)
    dp[1].enable_alu(AluOp.MULTIPLY, AluInp.PREV_ALU_OUT, D[4])  # q
    dp[1].pass_through_delay(5)
    dp[2].enable_alu(AluOp.SUBTRACT, D[5], AluInp.PREV_ALU_OUT)  # -q
    dp[2].enable_delay_from_src(DelayInp.PREV_ALU_OUT, 0)  # q
    for k in (3, 4, 5, 6, 7):
        dp[k].pass_through_alu()
        dp[k].pass_through_delay(0)
    u.enable_output(OutSel.DELAY_0, OutPath.WR0_LO)  # q
    u.enable_output(OutSel.ALU_OUT, OutPath.WR0_HI)  # -q
    return u


def _qim_reference(in0, in1, c0, c1, c2):
    b = np.asarray(in1).astype(np.float32)
    ss = np.asarray(c0, np.float32).reshape(-1, *([1] * (b.ndim - 1)))
    q = ss * (b[..., 0::2] - b[..., 1::2])
    out = np.empty_like(b)
    out[..., 0::2] = q
    out[..., 1::2] = -q
    return out


def _q0_reference(in0, in1, c0, c1, c2):
    a = in0.astype(np.float32)
    b = np.asarray(in1).astype(np.float32)
    out = np.empty_like(a)
    out[..., 0::2] = (a[..., 0::2] + a[..., 1::2]) + (b[..., 0::2] + b[..., 1::2])
    out[..., 1::2] = 0.0
    return out


def _q2_reference(in0, in1, c0, c1, c2):
    a = in0.astype(np.float32)
    b = np.asarray(in1).astype(np.float32)
    out = np.empty_like(a)
    out[..., 0::2] = (a[..., 0::2] + a[..., 1::2]) - (b[..., 0::2] + b[..., 1::2])
    out[..., 1::2] = 0.0
    return out


def _q13_reference(sub):
    def ref(in0, in1, c0, c1, c2):
        a = in0.astype(np.float32)
        b = np.asarray(in1).astype(np.float32)
        cc = np.asarray(c0, np.float32).reshape(-1, *([1] * (a.ndim - 1)))
        t2 = a[..., 0::2] - a[..., 1::2]
        u = b[..., 0::2] - b[..., 1::2]
        out = np.empty_like(a)
        out[..., 0::2] = t2 - cc * u if sub else t2 + cc * u
        out[..., 1::2] = 0.0
        return out

    return ref


# ---------------- registry ----------------


class RawDveOp:
    def __init__(self, name, mk_all, rd1_en, perf_max, reference):
        self.name = name
        self.subdim = False
        self.spec = Spec(body=Src0 * Src1 if rd1_en else Src0, reference=reference)
        self.rd1_en = rd1_en
        self.perf_max = perf_max
        self._mk = mk_all
        self._cache = {}

    def compile(self, ver):
        if ver in self._cache:
            return self._cache[ver]
        kw = self._mk()
        spec = DveOpSpec(
            name=self.name,
            opcode=dve_ops.get_dve_sub_opcode(self.name),
            perf_max=self.perf_max,
            rd1_en=self.rd1_en,
            **kw,
        )
        spec.validate(ver)
        self._cache[ver] = spec
        return spec


RAW_OPS = {}


def register_raw_ops():
    if RAW_OPS:
        return RAW_OPS
    defs = [
        RawDveOp(
            "CMUL_PACKED_ANT",
            lambda: dict(uops=[_cmul_uop()], uops_2x=[_cmul_uop()]),
            True,
            1,
            _cmul_reference,
        ),
        RawDveOp(
            "QUAD0_ANT",
            lambda: dict(uops=[_q02_uop(False)], uops_2x=[_q02_uop(False)]),
            True,
            1,
            _q0_reference,
        ),
        RawDveOp(
            "QUAD2_ANT",
            lambda: dict(uops=[_q02_uop(True)], uops_2x=[_q02_uop(True)]),
            True,
            1,
            _q2_reference,
        ),
        RawDveOp(
            "QUAD1_ANT",
            lambda: dict(uops=[_q13_uop(False)], uops_2x=[_q13_uop(False)]),
            True,
            1,
            _q13_reference(False),
        ),
        RawDveOp(
            "QUAD3_ANT",
            lambda: dict(uops=[_q13_uop(True)], uops_2x=[_q13_uop(True)]),
            True,
            1,
            _q13_reference(True),
        ),
        RawDveOp(
            "QIM_ANT",
            lambda: dict(uops=[_qim_uop()], uops_2x=[_qim_uop()]),
            True,
            1,
            _qim_reference,
        ),
        RawDveOp(
            "QA_ANT",
            lambda: dict(uops=_qab_uops(False), uops_2x=_qab_uops(False)),
            True,
            1,
            _qab_reference(False),
        ),
        RawDveOp(
            "QB_ANT",
            lambda: dict(uops=_qab_uops(True), uops_2x=_qab_uops(True)),
            True,
            1,
            _qab_reference(True),
        ),
    ]
    for op in defs:
        if op.name not in dve_ops._SUB_OPCODE_FOR_NAME:
            dve_ops.OPS.append(op)
            row = dve_ops._CUSTOM_DVE_ROW_BASE + len(dve_ops.OPS) - 1
            assert row < 0x20
            dve_ops._SUB_OPCODE_FOR_NAME[op.name] = row
            dve_ops.CUSTOM_DVE_SPECS[op.name] = op.spec
        RAW_OPS[op.name] = op
    return RAW_OPS


def emit_raw(nc, name, out, in0, in1=None, s0=None, s1=None):
    import concourse.bass_isa as bass_isa

    ops = register_raw_ops()
    op = ops[name]
    v = nc.vector
    if op.name not in nc.m.ant_custom_dve_ops:
        nc.m.ant_custom_dve_ops = sorted({*nc.m.ant_custom_dve_ops, op.name})
    shape = (
        bass_isa.CustomDveShape.STT
        if in1 is not None
        else bass_isa.CustomDveShape.TTSS
    )
    isa_opcode = nc.isa.Opcode[
        f"NEURON_ISA_TPB_OPCODE_CUSTOM_DVE_ANT_{shape.slot()}"
    ].value
    imm = mybir.ImmediateValue(dtype=mybir.dt.float32, value=0.0)
    s0a = v.lower_ap(s0, for_isa=True) if s0 is not None else imm
    s1a = v.lower_ap(s1, for_isa=True) if s1 is not None else imm
    ins = [v.lower_ap(in0, for_isa=True)]
    if in1 is not None:
        ins.append(v.lower_ap(in1, for_isa=True))
    ins += [s0a, s1a]
    return v.add_instruction(
        bass_isa.InstCustomDveAnt(
            name=nc.get_next_instruction_name(),
            op_name=op.name,
            rd1_en=op.rd1_en,
            subdim=0,
            imm2=0.0,
            shape=shape,
            row=dve_ops.get_dve_sub_opcode(op.name),
            isa_opcode=isa_opcode,
            ins=ins,
            outs=[v.lower_ap(out, for_isa=True)],
        )
    )


def patch_perf_bits(nc):
    ops = register_raw_ops()
    n = 0
    for fn in nc.m.functions:
        for blk in fn.blocks:
            for inst in blk.instructions:
                nm = getattr(inst, "op_name", None)
                if nm in ops:
                    bb = bytearray(bytes(inst.instr))
                    bb[36] |= ops[nm].perf_max << 6
                    inst.instr = bytes(bb)
                    n += 1
    return n


# ===================== kernel builder =====================


def build_fft_nc():
    register_raw_ops()
    nc = bacc.Bacc()

    xT = nc.dram_tensor("xT", [DSH, N], F16, kind="ExternalInput")
    # wT rows: [0:2048) = reduced phase r, [2048:4096) = |r| (host-prepared)
    wT = nc.dram_tensor("wT", [DSH, N], F32, kind="ExternalInput")
    wc = nc.dram_tensor("wc", [DSH, 2], F32, kind="ExternalInput")  # (rc, |rc|)
    outT = nc.dram_tensor("outT", [DSH, 2 * N], F16, kind="ExternalOutput")

    # const AP: pi/2 bias for the cos path
    HPI = float(np.float32(PI / 2))
    chp = nc.alloc_sbuf_tensor("const-f32-hpi", [128, 1], F32)
    nc.gpsimd.memset(chp.ap(), HPI)
    nc.const_aps.aps[(F32, HPI)] = chp.ap()
    nc.all_engine_barrier()

    with TileContext(nc) as tc:
        with (
            tc.tile_pool(name="xr", bufs=2) as xrpool,
            tc.tile_pool(name="xbuf", bufs=2) as xpool,
            tc.tile_pool(name="tmp", bufs=2) as tpool,
            tc.tile_pool(name="tw", bufs=2) as twpool,
            tc.tile_pool(name="ph", bufs=2) as ppool,
            tc.tile_pool(name="col", bufs=2) as colpool,
        ):
            for g in range(NGROUPS):
                rows = slice(g * 128, (g + 1) * 128)

                # ---- tiny stage-2 column phases first (unblocks quads) ----
                rc = colpool.tile([128, 2], F32, tag="rc")
                nc.sync.dma_start(rc[:], wc[rows, :])
                cols = colpool.tile([128, 2], F32, tag="cols")
                nc.scalar.activation(
                    cols[:, 0:1], rc[:, 1:2], AF.Sin, scale=-TWO_PI, bias=HPI
                )  # c = cos
                nc.scalar.activation(cols[:, 1:2], rc[:, 0:1], AF.Sin, scale=TWO_PI)

                # ---- phases arrive host-reduced: wT[p, 0:2048] = r in
                # [-0.5, 0.5] with sin(2pi*r) = sin(phi); wT[p, 2048:] = |r| ----
                r = ppool.tile([128, N], F32, tag="r")
                nc.sync.dma_start(r[:], wT[rows, :])

                # ---- interleaved twiddle packs: stage s at [2*half, 4*half).
                # All scalar work here is vector-independent: keep it ahead of
                # the im-scatter in the in-order scalar queue. ----
                pack = twpool.tile([128, 2 * N], F16, tag="pack")
                for s in range(3, LOGN + 1):
                    half = 1 << (s - 1)
                    stride = N >> s
                    src_im = r[:, 0 : N // 2 : stride]
                    src_re = r[:, N // 2 : N : stride]
                    nc.scalar.activation(
                        pack[:, 2 * half : 4 * half : 2],
                        src_re,
                        AF.Sin,
                        scale=-TWO_PI,
                        bias=HPI,
                    )
                    nc.scalar.activation(
                        pack[:, 2 * half + 1 : 4 * half : 2],
                        src_im,
                        AF.Sin,
                        scale=TWO_PI,
                    )

                # ---- x real plane (host pre-permuted fp16) ----
                xr = xrpool.tile([128, N], F16, tag="xplane")
                nc.sync.dma_start(xr[:], xT[rows, :])

                # ---- stages 1+2: fused quad ops -> packed complex x.
                # QIM first so its ACT im-scatter overlaps Q0/Q2. ----
                x = xpool.tile([128, 2 * N], F16, tag="x")
                xr4 = xr[:].rearrange("p (b f) -> p b f", f=4)
                src0 = xr4[:, :, 0:2]
                src1 = xr4[:, :, 2:4]
                x8 = x[:].rearrange("p (b f) -> p b f", f=8)
                imc = tpool.tile([128, N // 2], F16, tag="imc")
                imc2 = imc[:].rearrange("p (b f) -> p b f", f=2)
                emit_raw(nc, "QIM_ANT", imc2, src0, src1, s0=cols[:, 1:2])
                if USE_QAB:
                    emit_raw(
                        nc, "QA_ANT", x8[:, :, 0:4], src0, src1, s0=cols[:, 0:1]
                    )
                    emit_raw(
                        nc, "QB_ANT", x8[:, :, 4:8], src0, src1, s0=cols[:, 0:1]
                    )
                    # scatter ims: odd quads first (stage-3 CMUL reads them),
                    # even quads overlap the CMUL
                    x16 = x[:].rearrange("p (b f) -> p b f", f=16)
                    im4 = imc[:].rearrange("p (b f) -> p b f", f=4)
                    nc.scalar.activation(x16[:, :, 11:16:4], im4[:, :, 2:4], AF.Copy)
                    nc.scalar.activation(x16[:, :, 3:8:4], im4[:, :, 0:2], AF.Copy)
                else:
                    emit_raw(
                        nc, "QUAD1_ANT", x8[:, :, 2:4], src0, src1, s0=cols[:, 0:1]
                    )
                    emit_raw(
                        nc, "QUAD3_ANT", x8[:, :, 6:8], src0, src1, s0=cols[:, 0:1]
                    )
                    nc.scalar.activation(x8[:, :, 3:8:4], imc2, AF.Copy)
                    emit_raw(nc, "QUAD0_ANT", x8[:, :, 0:2], src0, src1)
                    emit_raw(nc, "QUAD2_ANT", x8[:, :, 4:6], src0, src1)

                t1 = tpool.tile([128, N], F16, tag="t1")  # packed cmul temp

                # ---- stages 3..11: packed generic. A 512-word slice of the
                # add pass runs on GpSimd: even-index blocks in the first
                # half -- next stage's CMUL (odd blocks) never reads them,
                # only the later sub/add do, so GpSimd overlaps fully. ----
                for s in range(3, LOGN):
                    step = 1 << s
                    half = step // 2
                    nb = N // step

                    xv = x[:].rearrange("p (b stc) -> p b stc", stc=2 * step)
                    top = xv[:, :, 0 : 2 * half]
                    bot = xv[:, :, 2 * half : 2 * step]
                    tw = (
                        pack[:, 2 * half : 4 * half]
                        .unsqueeze(1)
                        .broadcast_to([128, nb, 2 * half])
                    )
                    tv = t1[:, 0 : nb * 2 * half]
                    tvb = tv.rearrange("p (b h) -> p b h", h=2 * half)
                    if nb > 1:
                        tv = tvb
                    emit_raw(nc, "CMUL_PACKED_ANT", tv, bot, tw)
                    nc.vector.tensor_sub(bot, top, tv)
                    if not GP_OFFLOAD:
                        nc.vector.tensor_add(top, top, tv)
                    elif nb >= 4:
                        h2 = nb // 2
                        nc.gpsimd.tensor_add(
                            xv[:, 0:h2:2, 0 : 2 * half],
                            xv[:, 0:h2:2, 0 : 2 * half],
                            tvb[:, 0:h2:2],
                        )
                        nc.vector.tensor_add(
                            xv[:, 1:h2:2, 0 : 2 * half],
                            xv[:, 1:h2:2, 0 : 2 * half],
                            tvb[:, 1:h2:2],
                        )
                        nc.vector.tensor_add(
                            xv[:, h2:nb, 0 : 2 * half],
                            xv[:, h2:nb, 0 : 2 * half],
                            tvb[:, h2:nb],
                        )
                    else:
                        # s == 11: nb == 2; gpsimd takes the first half of
                        # block 0's top (consumed by stage 12's late ops)
                        nc.gpsimd.tensor_add(
                            xv[:, 0:1, 0:half],
                            xv[:, 0:1, 0:half],
                            tvb[:, 0:1, 0:half],
                        )
                        nc.vector.tensor_add(
                            xv[:, 0:1, half : 2 * half],
                            xv[:, 0:1, half : 2 * half],
                            tvb[:, 0:1, half : 2 * half],
                        )
                        nc.vector.tensor_add(
                            xv[:, 1:2, 0 : 2 * half],
                            xv[:, 1:2, 0 : 2 * half],
                            tvb[:, 1:2],
                        )

                # ---- stage 12 in two column chunks; outputs DMA straight to
                # HBM as packed fp16 ----
                Q = N // 2
                for c in range(2):
                    top_c = x[:, c * Q : c * Q + Q]
                    bot_c = x[:, N + c * Q : N + c * Q + Q]
                    tw_c = pack[:, N + c * Q : N + c * Q + Q]
                    tv = t1[:, 0:Q]
                    emit_raw(nc, "CMUL_PACKED_ANT", tv, bot_c, tw_c)
                    nc.vector.tensor_sub(bot_c, top_c, tv)
                    hq = 2 + c
                    nc.sync.dma_start(
                        outT[rows, hq * Q : (hq + 1) * Q], x[:, hq * Q : (hq + 1) * Q]
                    )
                    nc.vector.tensor_add(top_c, top_c, tv)
                    nc.sync.dma_start(
                        outT[rows, c * Q : (c + 1) * Q], x[:, c * Q : (c + 1) * Q]
                    )

    nc.compile()
    patch_perf_bits(nc)
    return nc


# ===================== host glue =====================

_PERM = None


def _perm():
    global _PERM
    if _PERM is None:
        _PERM = np.arange(N) ^ (N // 2)
    return _PERM


def make_core_inputs(x: np.ndarray, weights: np.ndarray, core: int):
    sl = slice(core * DSH, (core + 1) * DSH)
    xp = x[_perm()][:, sl]  # pre-permuted rows
    xT = np.ascontiguousarray(xp.T).astype(np.float16)
    w = weights[: N // 2, sl].astype(np.float64)
    k = -(1.0 / N) * np.arange(N // 2, dtype=np.float64)
    rr = w * k[:, None]
    rr -= np.rint(rr)
    rT = np.ascontiguousarray(rr.T).astype(np.float32)
    wT = np.concatenate([rT, np.abs(rT)], axis=1)
    wc = np.ascontiguousarray(wT[:, [1024, 2048 + 1024]])
    return {"xT": xT, "wT": wT, "wc": wc}


def assemble_output(core_outs):
    full = np.empty((N, N), dtype=np.complex64)
    for c, r in enumerate(core_outs):
        oc = r["outT"].astype(np.float32).view(np.complex64)
        full[:, c * DSH : (c + 1) * DSH] = oc.T
    return full


_NC_CACHE = None


def get_nc():
    global _NC_CACHE
    if _NC_CACHE is None:
        _NC_CACHE = build_fft_nc()
    return _NC_CACHE


def make_in_maps(x: np.ndarray, weights: np.ndarray):
    x = np.asarray(x, dtype=np.float32)
    weights = np.asarray(weights, dtype=np.float32)
    in_maps = [make_core_inputs(x, weights, c) for c in range(NCORES)]
    return in_maps


def run_on_hw(x, weights, **spmd_kwargs):
    nc = get_nc()
    in_maps = make_in_maps(x, weights)
    res = run_bass_kernel_spmd(nc, in_maps, core_ids=list(range(NCORES)), **spmd_kwargs)
    return assemble_output(res.results), res


def kernel(x: np.ndarray, weights: np.ndarray) -> np.ndarray:
    out, _ = run_on_hw(x, weights)
    return out
